# revision 21
# baseline (speedup 1.0000x reference)
"""Trainium2 Bass kernel for CausalSelfAttentionModern (GQA + RoPE + causal SDPA).

Sharding: tensor-parallel over heads across 8 NeuronCores.
Device d owns q-heads {2d, 2d+1} and kv-head d//2.
Each device computes its heads' attention plus its slice of the output
projection (row-parallel); the host sums the 8 partial outputs.

All matmuls run as float32r (full-rate fp32 mode on the PE array).
"""

import numpy as np
import concourse.bacc as bacc
import concourse.tile as tile
import concourse.mybir as mybir
from concourse.bass_utils import run_bass_kernel_spmd

F32 = mybir.dt.float32
F32R = mybir.dt.float32r
EXP = mybir.ActivationFunctionType.Exp

# hardcoded problem shapes
T = 2048          # sequence length
C = 2048          # embedding dim
DH = 128          # head dim
NH = 16           # query heads
NKV = 4           # kv heads
N_CORES = 8
HPD = NH // N_CORES  # q-heads per device = 2
ROPE_BASE = 10000.0
SCALE = 1.0 / np.sqrt(DH)

NQ = 4            # t-quarters for projection phase
TQ = T // NQ      # 512
NW = 4            # attention tq windows
TW = T // NW      # 512
NCT = C // 128    # 16 contraction tiles
NTC = T // 128    # 16 token chunks


def _emit(nc):
    xT = nc.dram_tensor("xT", [C, T], F32R, kind="ExternalInput").ap()
    wq = nc.dram_tensor("wq", [128, NCT * HPD * DH], F32R, kind="ExternalInput").ap()
    wk = nc.dram_tensor("wk", [128, NCT * DH], F32R, kind="ExternalInput").ap()
    wv = nc.dram_tensor("wv", [128, NCT * DH], F32R, kind="ExternalInput").ap()
    wp = nc.dram_tensor("wp", [128, HPD * C], F32R, kind="ExternalInput").ap()
    cosT = nc.dram_tensor("cosT", [128, T], F32, kind="ExternalInput").ap()
    sinR = nc.dram_tensor("sinR", [128, T], F32, kind="ExternalInput").ap()
    ones = nc.dram_tensor("ones", [128, 128], F32R, kind="ExternalInput").ap()
    ident = nc.dram_tensor("ident", [128, 128], F32, kind="ExternalInput").ap()
    out = nc.dram_tensor("out", [T, C], F32, kind="ExternalOutput").ap()

    with tile.TileContext(nc) as tc:
        with (
            tc.tile_pool(name="cst", bufs=1) as cst,
            tc.tile_pool(name="ps", bufs=8, space="PSUM") as ps,
        ):
            # persistent SBUF tensors (DMAs emitted at first-use points below)
            cos_sb = cst.tile([128, T], F32, tag="cos")
            sin_sb = cst.tile([128, T], F32, tag="sin")
            ones_sb = cst.tile([128, 128], F32R, tag="ones")
            id_sb = cst.tile([128, 128], F32, tag="ident")
            wp_sb = cst.tile([128, HPD * C], F32R, tag="wp")

            qt_sb = [cst.tile([128, T], F32R, tag=f"qt{m}", name=f"qt{m}")
                     for m in range(HPD)]
            kt_sb = cst.tile([128, T], F32R, tag="kt")
            vtp_pool = None  # vt quarter tiles come from the rope pool
            v_sb = cst.tile([128, NTC * DH], F32R, tag="v")
            yt_sb = [cst.tile([128, T], F32R, tag=f"yt{m}", name=f"yt{m}")
                     for m in range(HPD)]

            # ---------------- projections + rope, per t-quarter ----------------
            with (
                tc.tile_pool(name="wqkv", bufs=1) as wqkv,
                tc.tile_pool(name="xts", bufs=5) as xts,
                tc.tile_pool(name="rope", bufs=1) as rope,
            ):
                wq_sb = wqkv.tile([128, NCT * HPD * DH], F32R, tag="wq")
                wk_sb = wqkv.tile([128, NCT * DH], F32R, tag="wk")
                wv_sb = wqkv.tile([128, NCT * DH], F32R, tag="wv")
                # weights on the ACT ring: c-tile groups so sems fire early
                for a, b in [(0, 1), (1, 4), (4, 8), (8, 16)]:
                    q1 = HPD * DH
                    nc.scalar.dma_start(wq_sb[:, a * q1:b * q1], wq[:, a * q1:b * q1])
                    nc.scalar.dma_start(wk_sb[:, a * DH:b * DH], wk[:, a * DH:b * DH])
                    nc.scalar.dma_start(wv_sb[:, a * DH:b * DH], wv[:, a * DH:b * DH])

                xt_tiles = {}
                vt_tiles = {}

                def emit_xt_loads(qq):
                    # per half (8 c-tiles x 256 tokens) strided load
                    SW = TQ // 2
                    tsl = slice(qq * SW, (qq + 1) * SW)
                    for half in range(2):
                        xt = xts.tile([128, 8 * SW], F32R, tag="xt",
                                      name=f"xtq{qq}_{half}")
                        c0 = half * 8
                        splits = [(0, 4), (4, 8)] if (qq == 0 and half == 0) else [(0, 8)]
                        for a, b in splits:
                            nc.sync.dma_start(
                                xt[:, a * SW:b * SW].rearrange("p (ct t) -> p ct t", t=SW),
                                xT[(c0 + a) * 128:(c0 + b) * 128, tsl].rearrange(
                                    "(ct p) t -> p ct t", p=128))
                        xt_tiles[(qq, half)] = xt

                def emit_proj_quarter(q):
                    SW = TQ // 2
                    tsl = slice(q * TQ, (q + 1) * TQ)
                    pq = [ps.tile([128, TQ], F32, tag="ps", name=f"pq{q}_{m}")
                          for m in range(HPD)]
                    pk = ps.tile([128, TQ], F32, tag="ps", name=f"pk{q}")
                    pv = ps.tile([128, TQ], F32, tag="ps", name=f"pv{q}")
                    for s in range(2):
                        osl = slice(s * SW, (s + 1) * SW)
                        for ct in range(NCT):
                            xt = xt_tiles[(2 * q + s, ct // 8)]
                            xsl = slice((ct % 8) * SW, (ct % 8 + 1) * SW)
                            st = ct == 0
                            sp = ct == NCT - 1
                            for m in range(HPD):
                                nc.tensor.matmul(
                                    pq[m][:, osl],
                                    wq_sb[:, (ct * HPD + m) * DH:(ct * HPD + m + 1) * DH],
                                    xt[:, xsl], start=st, stop=sp)
                            nc.tensor.matmul(
                                pk[:, osl], wk_sb[:, ct * DH:(ct + 1) * DH],
                                xt[:, xsl], start=st, stop=sp)
                            nc.tensor.matmul(
                                pv[:, osl], wv_sb[:, ct * DH:(ct + 1) * DH],
                                xt[:, xsl], start=st, stop=sp)

                    if q == 0:
                        # constants needed from the rope/attention phases on
                        nc.scalar.dma_start(cos_sb[:], cosT[:])
                        nc.scalar.dma_start(sin_sb[:], sinR[:])
                        nc.scalar.dma_start(id_sb[:], ident[:])
                        nc.scalar.dma_start(ones_sb[:], ones[:])
                    if q == 2:
                        nc.scalar.dma_start(wp_sb[:], wp[:])

                    # rope: out = psum*cos + shift(psum)*sinR  (shift = rotate-half)
                    for psrc, dst in [(pq[0], qt_sb[0]), (pq[1], qt_sb[1]), (pk, kt_sb)]:
                        cr = rope.tile([128, TQ], F32, tag="crope")
                        nc.vector.tensor_mul(cr[:], psrc[:], cos_sb[:, tsl])
                        ur = rope.tile([128, TQ], F32, tag="urot")
                        nc.vector.tensor_mul(ur[0:64, :], psrc[64:128, :], sin_sb[0:64, tsl])
                        nc.vector.tensor_mul(ur[64:128, :], psrc[0:64, :], sin_sb[64:128, tsl])
                        nc.vector.tensor_add(dst[:, tsl], cr[:], ur[:])
                    # v: plain copy to SBUF (fp32, feeds PE transpose)
                    vt_q = rope.tile([128, TQ], F32, tag="vtq", name=f"vtq{q}")
                    nc.scalar.copy(vt_q[:], pv[:])
                    vt_tiles[q] = vt_q

                def emit_v_transpose(q):
                    # transpose V^T -> V for quarter q (4 token chunks)
                    pvt = ps.tile([128, TQ], F32, tag="ps", name=f"pvt{q}")
                    for j in range(4):
                        nc.tensor.transpose(
                            pvt[:, j * 128:(j + 1) * 128],
                            vt_tiles[q][:, j * 128:(j + 1) * 128],
                            id_sb[:])
                    nc.vector.tensor_copy(v_sb[:, q * TQ:(q + 1) * TQ], pvt[:])

                def emit_attn_window(tw0, twl):
                    wsl = slice(tw0, tw0 + twl)
                    nch = (tw0 + twl) // 128
                    w = tw0 // 128  # first diagonal chunk index
                    for h in range(HPD):
                        # phase 1: scores -> exp -> causal-zero, decoupled from pV
                        pts = []
                        for cc in range(nch):
                            # columns < rel are entirely above-diagonal for this
                            # chunk; skip them when a 256-wide tail still remains
                            rel = cc * 128 - tw0
                            n0 = 256 if (rel >= 256 and twl - 256 >= 256) else 0
                            csl = slice(n0, twl)
                            sc_ps = ps.tile([128, twl], F32, tag="ps",
                                            name=f"sc{w}_{h}_{cc}")
                            nc.tensor.matmul(
                                sc_ps[:, csl],
                                kt_sb[:, cc * 128:(cc + 1) * 128],
                                qt_sb[h][:, tw0 + n0:tw0 + twl], start=True, stop=True)
                            pt = ptp.tile([128, twl], F32R, tag="pt",
                                          name=f"pt{w}_{h}_{cc}")
                            nc.scalar.activation(pt[:, csl], sc_ps[:, csl], EXP,
                                                 scale=float(SCALE))
                            if cc >= w:
                                # zero strictly-above-diagonal: keep where tq >= tk
                                nc.gpsimd.affine_select(
                                    out=pt[:, csl], in_=pt[:, csl],
                                    pattern=[[1, twl - n0]],
                                    compare_op=mybir.AluOpType.is_ge, fill=0.0,
                                    base=tw0 + n0 - cc * 128, channel_multiplier=-1)
                            pts.append((pt, n0))
                        # phase 2: y^T += V^T-chunks @ probs, sums via ones-matmul
                        y_ps = ps.tile([128, twl], F32, tag="ps", name=f"y{w}_{h}")
                        s_ps = ps.tile([128, twl], F32, tag="ps", name=f"s{w}_{h}")
                        for cc in range(nch):
                            st = cc == 0
                            sp = cc == nch - 1
                            pt, n0 = pts[cc]
                            csl = slice(n0, twl)
                            nc.tensor.matmul(
                                y_ps[:, csl], v_sb[:, cc * DH:(cc + 1) * DH],
                                pt[:, csl], start=st, stop=sp)
                            nc.tensor.matmul(
                                s_ps[:, csl], ones_sb[:], pt[:, csl],
                                start=st, stop=sp)
                        rc = rcp.tile([128, twl], F32, tag="rc", name=f"rc{w}_{h}")
                        nc.vector.reciprocal(rc[:], s_ps[:])
                        nc.vector.tensor_mul(yt_sb[h][:, wsl], y_ps[:], rc[:])

                def emit_outproj_window(tw0, twl, split_out=False):
                    for j in range(twl // 128):
                        t0 = tw0 + j * 128
                        po = [ps.tile([128, 512], F32, tag="ps",
                                      name=f"po{t0}_{e}") for e in range(4)]
                        for k in range(HPD):
                            for e in range(4):
                                nc.tensor.matmul(
                                    po[e][:],
                                    yt_sb[k][:, t0:t0 + 128],
                                    wp_sb[:, k * C + e * 512:k * C + (e + 1) * 512],
                                    start=(k == 0), stop=(k == HPD - 1))
                        ost = ostp.tile([128, C], F32, tag="ost", name=f"ost{t0}")
                        if split_out:
                            for e in range(4):
                                esl = slice(e * 512, (e + 1) * 512)
                                nc.any.tensor_copy(ost[:, esl], po[e][:])
                                nc.sync.dma_start(out[t0:t0 + 128, esl], ost[:, esl])
                        else:
                            for e in range(4):
                                nc.any.tensor_copy(ost[:, e * 512:(e + 1) * 512], po[e][:])
                            nc.sync.dma_start(out[t0:t0 + 128, :], ost[:])

                with (
                    tc.tile_pool(name="pt", bufs=14) as ptp,
                    tc.tile_pool(name="rc", bufs=2) as rcp,
                    tc.tile_pool(name="ost", bufs=2) as ostp,
                ):
                    for qq in range(2 * NQ):
                        emit_xt_loads(qq)
                    for q in range(NQ):
                        emit_proj_quarter(q)
                        if q >= 2:
                            emit_outproj_window((q - 2) * TW, TW)
                        if q >= 1:
                            emit_v_transpose(q - 1)
                            emit_attn_window((q - 1) * TW, TW)
                    emit_v_transpose(NQ - 1)
                    emit_outproj_window((NQ - 2) * TW, TW)
                    emit_attn_window(3 * TW, TW // 2)
                    emit_outproj_window(3 * TW, TW // 2)
                    emit_attn_window(3 * TW + TW // 2, TW // 2)
                    emit_outproj_window(3 * TW + TW // 2, TW // 2)

    nc.compile()
    return nc


_CACHE = {}


def _get_module():
    if "nc" not in _CACHE:
        nc = bacc.Bacc("TRN2", target_bir_lowering=False, debug=False)
        _CACHE["nc"] = _emit(nc)
    return _CACHE["nc"]


def _host_constants():
    if "consts" in _CACHE:
        return _CACHE["consts"]
    inv_freq = 1.0 / (ROPE_BASE ** (np.arange(0, DH, 2, dtype=np.float64) / DH))
    ang = np.outer(np.arange(T, dtype=np.float64), inv_freq)      # (T, 64)
    emb = np.concatenate([ang, ang], axis=-1)                     # (T, 128)
    cos = np.cos(emb).astype(np.float32)                          # (T, 128)
    sin = np.sin(emb).astype(np.float32)
    cosT = np.ascontiguousarray(cos.T)                            # (128, T)
    sinT = np.ascontiguousarray(sin.T)
    sign = np.where(np.arange(DH) < DH // 2, -1.0, 1.0).astype(np.float32)
    sinR = np.ascontiguousarray(sinT * sign[:, None])
    ones = np.ones((128, 128), dtype=np.float32)
    ident = np.eye(128, dtype=np.float32)
    _CACHE["consts"] = (cosT, sinR, ones, ident)
    return _CACHE["consts"]


def kernel(x, wq, wk, wv, wproj):
    x = np.asarray(x, dtype=np.float32)
    wq = np.asarray(wq, dtype=np.float32)
    wk = np.asarray(wk, dtype=np.float32)
    wv = np.asarray(wv, dtype=np.float32)
    wproj = np.asarray(wproj, dtype=np.float32)

    nc = _get_module()
    cosT, sinR, ones, ident = _host_constants()
    xT = np.ascontiguousarray(x[0].T)                             # (C, T)

    in_maps = []
    for d in range(N_CORES):
        h0 = HPD * d
        g = d // 2
        # wq columns for heads h0..h0+HPD-1 -> [128, NCT*HPD*DH] (c-tile major)
        wq_d = wq[:, h0 * DH:(h0 + HPD) * DH]                     # (C, HPD*DH)
        wq_l = np.ascontiguousarray(
            wq_d.reshape(NCT, 128, HPD * DH).transpose(1, 0, 2).reshape(128, -1))
        wk_d = wk[:, g * DH:(g + 1) * DH]
        wk_l = np.ascontiguousarray(
            wk_d.reshape(NCT, 128, DH).transpose(1, 0, 2).reshape(128, -1))
        wv_d = wv[:, g * DH:(g + 1) * DH]
        wv_l = np.ascontiguousarray(
            wv_d.reshape(NCT, 128, DH).transpose(1, 0, 2).reshape(128, -1))
        # wproj rows for our heads -> [128, HPD*C] (head-major free dim)
        wp_d = wproj[h0 * DH:(h0 + HPD) * DH, :]                  # (HPD*DH, C)
        wp_l = np.ascontiguousarray(
            wp_d.reshape(HPD, 128, C).transpose(1, 0, 2).reshape(128, -1))
        in_maps.append({
            "xT": xT, "wq": wq_l, "wk": wk_l, "wv": wv_l, "wp": wp_l,
            "cosT": cosT, "sinR": sinR, "ones": ones, "ident": ident,
        })

    res = run_bass_kernel_spmd(nc, in_maps, core_ids=list(range(N_CORES)))
    acc = res.results[0]["out"].astype(np.float32)
    for d in range(1, N_CORES):
        acc = acc + res.results[d]["out"].astype(np.float32)
    return acc.reshape(1, T, C)


# revision 24
# speedup vs baseline: 1.0833x; 1.0833x over previous
"""Trainium2 Bass kernel for CausalSelfAttentionModern (GQA + RoPE + causal SDPA).

Sharding: tensor-parallel over heads across 8 NeuronCores.
Device d owns q-heads {2d, 2d+1} and kv-head d//2.
Each device computes its heads' attention plus its slice of the output
projection (row-parallel); the host sums the 8 partial outputs.

All matmuls run as float32r (full-rate fp32 mode on the PE array).
"""

import numpy as np
import concourse.bacc as bacc
import concourse.tile as tile
import concourse.mybir as mybir
from concourse.bass_utils import run_bass_kernel_spmd

F32 = mybir.dt.float32
F32R = mybir.dt.float32r
EXP = mybir.ActivationFunctionType.Exp

# hardcoded problem shapes
T = 2048          # sequence length
C = 2048          # embedding dim
DH = 128          # head dim
NH = 16           # query heads
NKV = 4           # kv heads
N_CORES = 8
HPD = NH // N_CORES  # q-heads per device = 2
ROPE_BASE = 10000.0
SCALE = 1.0 / np.sqrt(DH)

NQ = 4            # t-quarters for projection phase
TQ = T // NQ      # 512
NW = 4            # attention tq windows
TW = T // NW      # 512
NCT = C // 128    # 16 contraction tiles
NTC = T // 128    # 16 token chunks


def _emit(nc):
    xT = nc.dram_tensor("xT", [C, T], F32R, kind="ExternalInput").ap()
    wq = nc.dram_tensor("wq", [128, NCT * HPD * DH], F32R, kind="ExternalInput").ap()
    wk = nc.dram_tensor("wk", [128, NCT * DH], F32R, kind="ExternalInput").ap()
    wv = nc.dram_tensor("wv", [128, NCT * DH], F32R, kind="ExternalInput").ap()
    wp = nc.dram_tensor("wp", [128, HPD * C], F32R, kind="ExternalInput").ap()
    cosT = nc.dram_tensor("cosT", [128, T], F32, kind="ExternalInput").ap()
    sinR = nc.dram_tensor("sinR", [128, T], F32, kind="ExternalInput").ap()
    ones = nc.dram_tensor("ones", [128, 128], F32R, kind="ExternalInput").ap()
    ident = nc.dram_tensor("ident", [128, 128], F32, kind="ExternalInput").ap()
    out = nc.dram_tensor("out", [T, C], F32, kind="ExternalOutput").ap()

    with tile.TileContext(nc) as tc:
        with (
            tc.tile_pool(name="cst", bufs=1) as cst,
            tc.tile_pool(name="ps", bufs=8, space="PSUM") as ps,
        ):
            # persistent SBUF tensors (DMAs emitted at first-use points below)
            cos_sb = cst.tile([128, T], F32, tag="cos")
            sin_sb = cst.tile([128, T], F32, tag="sin")
            ones_sb = cst.tile([128, 128], F32R, tag="ones")
            id_sb = cst.tile([128, 128], F32, tag="ident")
            wp_sb = cst.tile([128, HPD * C], F32R, tag="wp")

            qt_sb = [cst.tile([128, T], F32R, tag=f"qt{m}", name=f"qt{m}")
                     for m in range(HPD)]
            kt_sb = cst.tile([128, T], F32R, tag="kt")
            vtp_pool = None  # vt quarter tiles come from the rope pool
            v_sb = cst.tile([128, NTC * DH], F32R, tag="v")
            yt_sb = [cst.tile([128, T], F32R, tag=f"yt{m}", name=f"yt{m}")
                     for m in range(HPD)]

            # ---------------- projections + rope, per t-quarter ----------------
            with (
                tc.tile_pool(name="wqkv", bufs=1) as wqkv,
                tc.tile_pool(name="xts", bufs=5) as xts,
                tc.tile_pool(name="rope", bufs=1) as rope,
            ):
                wq_sb = wqkv.tile([128, NCT * HPD * DH], F32R, tag="wq")
                wk_sb = wqkv.tile([128, NCT * DH], F32R, tag="wk")
                wv_sb = wqkv.tile([128, NCT * DH], F32R, tag="wv")
                # weights on the ACT ring: c-tile groups so sems fire early
                for a, b in [(0, 1), (1, 4), (4, 8), (8, 16)]:
                    q1 = HPD * DH
                    nc.scalar.dma_start(wq_sb[:, a * q1:b * q1], wq[:, a * q1:b * q1])
                    nc.scalar.dma_start(wk_sb[:, a * DH:b * DH], wk[:, a * DH:b * DH])
                    nc.scalar.dma_start(wv_sb[:, a * DH:b * DH], wv[:, a * DH:b * DH])

                xt_tiles = {}
                vt_tiles = {}

                def emit_xt_loads(qq):
                    # per half (8 c-tiles x 256 tokens) strided load
                    SW = TQ // 2
                    tsl = slice(qq * SW, (qq + 1) * SW)
                    for half in range(2):
                        xt = xts.tile([128, 8 * SW], F32R, tag="xt",
                                      name=f"xtq{qq}_{half}")
                        c0 = half * 8
                        splits = [(0, 4), (4, 8)] if (qq == 0 and half == 0) else [(0, 8)]
                        for a, b in splits:
                            nc.sync.dma_start(
                                xt[:, a * SW:b * SW].rearrange("p (ct t) -> p ct t", t=SW),
                                xT[(c0 + a) * 128:(c0 + b) * 128, tsl].rearrange(
                                    "(ct p) t -> p ct t", p=128))
                        xt_tiles[(qq, half)] = xt

                def emit_proj_subq(i, mid_fn=None):
                    # one 256-token sub-quarter: psum tiles complete before rope
                    SW = TQ // 2
                    tsl = slice(i * SW, (i + 1) * SW)
                    pq = [ps.tile([128, SW], F32, tag="ps", name=f"pq{i}_{m}")
                          for m in range(HPD)]
                    pk = ps.tile([128, SW], F32, tag="ps", name=f"pk{i}")
                    pv = ps.tile([128, SW], F32, tag="ps", name=f"pv{i}")
                    for ct in range(NCT):
                        if ct == 8 and mid_fn is not None:
                            mid_fn()
                        xt = xt_tiles[(i, ct // 8)]
                        xsl = slice((ct % 8) * SW, (ct % 8 + 1) * SW)
                        st = ct == 0
                        sp = ct == NCT - 1
                        for m in range(HPD):
                            nc.tensor.matmul(
                                pq[m][:],
                                wq_sb[:, (ct * HPD + m) * DH:(ct * HPD + m + 1) * DH],
                                xt[:, xsl], start=st, stop=sp)
                        nc.tensor.matmul(
                            pk[:], wk_sb[:, ct * DH:(ct + 1) * DH],
                            xt[:, xsl], start=st, stop=sp)
                        nc.tensor.matmul(
                            pv[:], wv_sb[:, ct * DH:(ct + 1) * DH],
                            xt[:, xsl], start=st, stop=sp)

                    if i == 0:
                        # constants needed from the rope/attention phases on
                        nc.scalar.dma_start(cos_sb[:], cosT[:])
                        nc.scalar.dma_start(sin_sb[:], sinR[:])
                        nc.scalar.dma_start(id_sb[:], ident[:])
                        nc.scalar.dma_start(ones_sb[:], ones[:])
                    if i == 1:
                        nc.scalar.dma_start(wp_sb[:], wp[:])

                    # rope: out = psum*cos + shift(psum)*sinR  (shift = rotate-half)
                    for psrc, dst in [(pq[0], qt_sb[0]), (pq[1], qt_sb[1]), (pk, kt_sb)]:
                        cr = rope.tile([128, SW], F32, tag="crope")
                        nc.vector.tensor_mul(cr[:], psrc[:], cos_sb[:, tsl])
                        ur = rope.tile([128, SW], F32, tag="urot")
                        nc.vector.tensor_mul(ur[0:64, :], psrc[64:128, :], sin_sb[0:64, tsl])
                        nc.vector.tensor_mul(ur[64:128, :], psrc[0:64, :], sin_sb[64:128, tsl])
                        nc.vector.tensor_add(dst[:, tsl], cr[:], ur[:])
                    # v: plain copy to SBUF (fp32, feeds PE transpose)
                    vt_q = rope.tile([128, SW], F32, tag="vtq", name=f"vtq{i}")
                    nc.scalar.copy(vt_q[:], pv[:])
                    vt_tiles[i] = vt_q

                def emit_v_transpose(i):
                    # transpose V^T -> V for sub-quarter i (2 token chunks)
                    SW = TQ // 2
                    pvt = ps.tile([128, SW], F32, tag="ps", name=f"pvt{i}")
                    for j in range(2):
                        nc.tensor.transpose(
                            pvt[:, j * 128:(j + 1) * 128],
                            vt_tiles[i][:, j * 128:(j + 1) * 128],
                            id_sb[:])
                    nc.vector.tensor_copy(v_sb[:, i * SW:(i + 1) * SW], pvt[:])

                def emit_attn_head(tw0, twl, h):
                    wsl = slice(tw0, tw0 + twl)
                    nch = (tw0 + twl) // 128
                    w = tw0 // 128  # first diagonal chunk index
                    if True:
                        # phase 1: scores -> exp -> causal-zero, decoupled from pV
                        pts = []
                        for cc in range(nch):
                            # columns < rel are entirely above-diagonal for this
                            # chunk; skip them when a 256-wide tail still remains
                            rel = cc * 128 - tw0
                            n0 = 256 if (rel >= 256 and twl - 256 >= 256) else 0
                            csl = slice(n0, twl)
                            sc_ps = ps.tile([128, twl], F32, tag="ps",
                                            name=f"sc{w}_{h}_{cc}")
                            nc.tensor.matmul(
                                sc_ps[:, csl],
                                kt_sb[:, cc * 128:(cc + 1) * 128],
                                qt_sb[h][:, tw0 + n0:tw0 + twl], start=True, stop=True)
                            pt = ptp.tile([128, twl], F32R, tag="pt",
                                          name=f"pt{w}_{h}_{cc}")
                            nc.scalar.activation(pt[:, csl], sc_ps[:, csl], EXP,
                                                 scale=float(SCALE))
                            if cc >= w:
                                # zero strictly-above-diagonal: keep where tq >= tk
                                nc.gpsimd.affine_select(
                                    out=pt[:, csl], in_=pt[:, csl],
                                    pattern=[[1, twl - n0]],
                                    compare_op=mybir.AluOpType.is_ge, fill=0.0,
                                    base=tw0 + n0 - cc * 128, channel_multiplier=-1)
                            pts.append((pt, n0))
                        # phase 2: y^T += V^T-chunks @ probs, sums via ones-matmul
                        y_ps = ps.tile([128, twl], F32, tag="ps", name=f"y{w}_{h}")
                        s_ps = ps.tile([128, twl], F32, tag="ps", name=f"s{w}_{h}")
                        for cc in range(nch):
                            st = cc == 0
                            sp = cc == nch - 1
                            pt, n0 = pts[cc]
                            csl = slice(n0, twl)
                            nc.tensor.matmul(
                                y_ps[:, csl], v_sb[:, cc * DH:(cc + 1) * DH],
                                pt[:, csl], start=st, stop=sp)
                            nc.tensor.matmul(
                                s_ps[:, csl], ones_sb[:], pt[:, csl],
                                start=st, stop=sp)
                        rc = rcp.tile([128, twl], F32, tag="rc", name=f"rc{w}_{h}")
                        nc.vector.reciprocal(rc[:], s_ps[:])
                        nc.vector.tensor_mul(yt_sb[h][:, wsl], y_ps[:], rc[:])

                def emit_outproj_window(tw0, twl, split_out=False):
                    for j in range(twl // 128):
                        t0 = tw0 + j * 128
                        po = [ps.tile([128, 512], F32, tag="ps",
                                      name=f"po{t0}_{e}") for e in range(4)]
                        for k in range(HPD):
                            for e in range(4):
                                nc.tensor.matmul(
                                    po[e][:],
                                    yt_sb[k][:, t0:t0 + 128],
                                    wp_sb[:, k * C + e * 512:k * C + (e + 1) * 512],
                                    start=(k == 0), stop=(k == HPD - 1))
                        ost = ostp.tile([128, C], F32, tag="ost", name=f"ost{t0}")
                        if split_out:
                            for e in range(4):
                                esl = slice(e * 512, (e + 1) * 512)
                                nc.any.tensor_copy(ost[:, esl], po[e][:])
                                nc.sync.dma_start(out[t0:t0 + 128, esl], ost[:, esl])
                        else:
                            for e in range(4):
                                nc.any.tensor_copy(ost[:, e * 512:(e + 1) * 512], po[e][:])
                            nc.sync.dma_start(out[t0:t0 + 128, :], ost[:])

                with (
                    tc.tile_pool(name="pt", bufs=17) as ptp,
                    tc.tile_pool(name="rc", bufs=2) as rcp,
                    tc.tile_pool(name="ost", bufs=3) as ostp,
                ):
                    SW = TQ // 2
                    for qq in range(2 * NQ):
                        emit_xt_loads(qq)
                    for i in range(2 * NQ):
                        if i >= 1:
                            w0 = (i - 1) * SW

                            def mid(w0=w0, i=i):
                                emit_v_transpose(i - 1)
                                emit_attn_head(w0, SW, 0)
                        else:
                            mid = None
                        emit_proj_subq(i, mid_fn=mid)
                        if i >= 1:
                            emit_attn_head((i - 1) * SW, SW, 1)
                        if i >= 2:
                            emit_outproj_window((i - 2) * SW, SW)
                    emit_v_transpose(2 * NQ - 1)
                    emit_attn_head((2 * NQ - 1) * SW, SW, 0)
                    emit_outproj_window((2 * NQ - 2) * SW, SW, split_out=True)
                    emit_attn_head((2 * NQ - 1) * SW, SW, 1)
                    emit_outproj_window((2 * NQ - 1) * SW, SW, split_out=True)

    nc.compile()
    return nc


_CACHE = {}


def _get_module():
    if "nc" not in _CACHE:
        nc = bacc.Bacc("TRN2", target_bir_lowering=False, debug=False)
        _CACHE["nc"] = _emit(nc)
    return _CACHE["nc"]


def _host_constants():
    if "consts" in _CACHE:
        return _CACHE["consts"]
    inv_freq = 1.0 / (ROPE_BASE ** (np.arange(0, DH, 2, dtype=np.float64) / DH))
    ang = np.outer(np.arange(T, dtype=np.float64), inv_freq)      # (T, 64)
    emb = np.concatenate([ang, ang], axis=-1)                     # (T, 128)
    cos = np.cos(emb).astype(np.float32)                          # (T, 128)
    sin = np.sin(emb).astype(np.float32)
    cosT = np.ascontiguousarray(cos.T)                            # (128, T)
    sinT = np.ascontiguousarray(sin.T)
    sign = np.where(np.arange(DH) < DH // 2, -1.0, 1.0).astype(np.float32)
    sinR = np.ascontiguousarray(sinT * sign[:, None])
    ones = np.ones((128, 128), dtype=np.float32)
    ident = np.eye(128, dtype=np.float32)
    _CACHE["consts"] = (cosT, sinR, ones, ident)
    return _CACHE["consts"]


def kernel(x, wq, wk, wv, wproj):
    x = np.asarray(x, dtype=np.float32)
    wq = np.asarray(wq, dtype=np.float32)
    wk = np.asarray(wk, dtype=np.float32)
    wv = np.asarray(wv, dtype=np.float32)
    wproj = np.asarray(wproj, dtype=np.float32)

    nc = _get_module()
    cosT, sinR, ones, ident = _host_constants()
    xT = np.ascontiguousarray(x[0].T)                             # (C, T)

    in_maps = []
    for d in range(N_CORES):
        h0 = HPD * d
        g = d // 2
        # wq columns for heads h0..h0+HPD-1 -> [128, NCT*HPD*DH] (c-tile major)
        wq_d = wq[:, h0 * DH:(h0 + HPD) * DH]                     # (C, HPD*DH)
        wq_l = np.ascontiguousarray(
            wq_d.reshape(NCT, 128, HPD * DH).transpose(1, 0, 2).reshape(128, -1))
        wk_d = wk[:, g * DH:(g + 1) * DH]
        wk_l = np.ascontiguousarray(
            wk_d.reshape(NCT, 128, DH).transpose(1, 0, 2).reshape(128, -1))
        wv_d = wv[:, g * DH:(g + 1) * DH]
        wv_l = np.ascontiguousarray(
            wv_d.reshape(NCT, 128, DH).transpose(1, 0, 2).reshape(128, -1))
        # wproj rows for our heads -> [128, HPD*C] (head-major free dim)
        wp_d = wproj[h0 * DH:(h0 + HPD) * DH, :]                  # (HPD*DH, C)
        wp_l = np.ascontiguousarray(
            wp_d.reshape(HPD, 128, C).transpose(1, 0, 2).reshape(128, -1))
        in_maps.append({
            "xT": xT, "wq": wq_l, "wk": wk_l, "wv": wv_l, "wp": wp_l,
            "cosT": cosT, "sinR": sinR, "ones": ones, "ident": ident,
        })

    res = run_bass_kernel_spmd(nc, in_maps, core_ids=list(range(N_CORES)))
    acc = res.results[0]["out"].astype(np.float32)
    for d in range(1, N_CORES):
        acc = acc + res.results[d]["out"].astype(np.float32)
    return acc.reshape(1, T, C)


# revision 28
# speedup vs baseline: 1.0836x; 1.0002x over previous
"""Trainium2 Bass kernel for CausalSelfAttentionModern (GQA + RoPE + causal SDPA).

Sharding: tensor-parallel over heads across 8 NeuronCores.
Device d owns q-heads {2d, 2d+1} and kv-head d//2.
Each device computes its heads' attention plus its slice of the output
projection (row-parallel); the host sums the 8 partial outputs.

All matmuls run as float32r (full-rate fp32 mode on the PE array).
"""

import numpy as np
import concourse.bacc as bacc
import concourse.tile as tile
import concourse.mybir as mybir
from concourse.bass_utils import run_bass_kernel_spmd

F32 = mybir.dt.float32
F32R = mybir.dt.float32r
EXP = mybir.ActivationFunctionType.Exp

# hardcoded problem shapes
T = 2048          # sequence length
C = 2048          # embedding dim
DH = 128          # head dim
NH = 16           # query heads
NKV = 4           # kv heads
N_CORES = 8
HPD = NH // N_CORES  # q-heads per device = 2
ROPE_BASE = 10000.0
SCALE = 1.0 / np.sqrt(DH)

NQ = 4            # t-quarters for projection phase
TQ = T // NQ      # 512
NW = 4            # attention tq windows
TW = T // NW      # 512
NCT = C // 128    # 16 contraction tiles
NTC = T // 128    # 16 token chunks


def _emit(nc):
    xT = nc.dram_tensor("xT", [C, T], F32R, kind="ExternalInput").ap()
    wq = nc.dram_tensor("wq", [128, NCT * HPD * DH], F32R, kind="ExternalInput").ap()
    wk = nc.dram_tensor("wk", [128, NCT * DH], F32R, kind="ExternalInput").ap()
    wv = nc.dram_tensor("wv", [128, NCT * DH], F32R, kind="ExternalInput").ap()
    wp = nc.dram_tensor("wp", [128, HPD * C], F32R, kind="ExternalInput").ap()
    cosT = nc.dram_tensor("cosT", [128, T], F32, kind="ExternalInput").ap()
    sinR = nc.dram_tensor("sinR", [128, T], F32, kind="ExternalInput").ap()
    ones = nc.dram_tensor("ones", [128, 128], F32R, kind="ExternalInput").ap()
    ident = nc.dram_tensor("ident", [128, 128], F32, kind="ExternalInput").ap()
    out = nc.dram_tensor("out", [T, C], F32, kind="ExternalOutput").ap()

    with tile.TileContext(nc) as tc:
        with (
            tc.tile_pool(name="cst", bufs=1) as cst,
            tc.tile_pool(name="ps", bufs=8, space="PSUM") as ps,
        ):
            # persistent SBUF tensors (DMAs emitted at first-use points below)
            cos_sb = cst.tile([128, T], F32, tag="cos")
            sin_sb = cst.tile([128, T], F32, tag="sin")
            ones_sb = cst.tile([128, 128], F32R, tag="ones")
            id_sb = cst.tile([128, 128], F32, tag="ident")
            wp_sb = cst.tile([128, HPD * C], F32R, tag="wp")

            qt_sb = [cst.tile([128, T], F32R, tag=f"qt{m}", name=f"qt{m}")
                     for m in range(HPD)]
            kt_sb = cst.tile([128, T], F32R, tag="kt")
            vtp_pool = None  # vt quarter tiles come from the rope pool
            v_sb = cst.tile([128, NTC * DH], F32R, tag="v")
            yt_sb = [cst.tile([128, T], F32R, tag=f"yt{m}", name=f"yt{m}")
                     for m in range(HPD)]

            # ---------------- projections + rope, per t-quarter ----------------
            with (
                tc.tile_pool(name="wqkv", bufs=1) as wqkv,
                tc.tile_pool(name="xts", bufs=5) as xts,
                tc.tile_pool(name="rope", bufs=1) as rope,
            ):
                wq_sb = wqkv.tile([128, NCT * HPD * DH], F32R, tag="wq")
                wk_sb = wqkv.tile([128, NCT * DH], F32R, tag="wk")
                wv_sb = wqkv.tile([128, NCT * DH], F32R, tag="wv")
                # weights on the ACT ring: c-tile groups so sems fire early
                for a, b in [(0, 1), (1, 4), (4, 8), (8, 16)]:
                    q1 = HPD * DH
                    nc.scalar.dma_start(wq_sb[:, a * q1:b * q1], wq[:, a * q1:b * q1])
                    nc.scalar.dma_start(wk_sb[:, a * DH:b * DH], wk[:, a * DH:b * DH])
                    nc.scalar.dma_start(wv_sb[:, a * DH:b * DH], wv[:, a * DH:b * DH])

                xt_tiles = {}
                vt_tiles = {}

                def emit_xt_loads(qq):
                    # per half (8 c-tiles x 256 tokens) strided load
                    SW = TQ // 2
                    tsl = slice(qq * SW, (qq + 1) * SW)
                    for half in range(2):
                        xt = xts.tile([128, 8 * SW], F32R, tag="xt",
                                      name=f"xtq{qq}_{half}")
                        c0 = half * 8
                        splits = [(0, 4), (4, 8)] if (qq == 0 and half == 0) else [(0, 8)]
                        for a, b in splits:
                            nc.sync.dma_start(
                                xt[:, a * SW:b * SW].rearrange("p (ct t) -> p ct t", t=SW),
                                xT[(c0 + a) * 128:(c0 + b) * 128, tsl].rearrange(
                                    "(ct p) t -> p ct t", p=128))
                        xt_tiles[(qq, half)] = xt

                def emit_proj_subq(i, mid_fn=None, hook_ct=8):
                    # one 256-token sub-quarter: psum tiles complete before rope
                    SW = TQ // 2
                    tsl = slice(i * SW, (i + 1) * SW)
                    pq = [ps.tile([128, SW], F32, tag="ps", name=f"pq{i}_{m}")
                          for m in range(HPD)]
                    pk = ps.tile([128, SW], F32, tag="ps", name=f"pk{i}")
                    pv = ps.tile([128, SW], F32, tag="ps", name=f"pv{i}")
                    for ct in range(NCT):
                        if ct == hook_ct and mid_fn is not None:
                            mid_fn()
                        xt = xt_tiles[(i, ct // 8)]
                        xsl = slice((ct % 8) * SW, (ct % 8 + 1) * SW)
                        st = ct == 0
                        sp = ct == NCT - 1
                        for m in range(HPD):
                            nc.tensor.matmul(
                                pq[m][:],
                                wq_sb[:, (ct * HPD + m) * DH:(ct * HPD + m + 1) * DH],
                                xt[:, xsl], start=st, stop=sp)
                        nc.tensor.matmul(
                            pk[:], wk_sb[:, ct * DH:(ct + 1) * DH],
                            xt[:, xsl], start=st, stop=sp)
                        nc.tensor.matmul(
                            pv[:], wv_sb[:, ct * DH:(ct + 1) * DH],
                            xt[:, xsl], start=st, stop=sp)

                    if i == 0:
                        # constants needed from the rope/attention phases on
                        nc.scalar.dma_start(cos_sb[:], cosT[:])
                        nc.scalar.dma_start(sin_sb[:], sinR[:])
                        nc.scalar.dma_start(id_sb[:], ident[:])
                        nc.scalar.dma_start(ones_sb[:], ones[:])
                    if i == 1:
                        nc.scalar.dma_start(wp_sb[:], wp[:])

                    # rope: out = psum*cos + shift(psum)*sinR  (shift = rotate-half)
                    for psrc, dst in [(pq[0], qt_sb[0]), (pq[1], qt_sb[1]), (pk, kt_sb)]:
                        cr = rope.tile([128, SW], F32, tag="crope")
                        nc.vector.tensor_mul(cr[:], psrc[:], cos_sb[:, tsl])
                        ur = rope.tile([128, SW], F32, tag="urot")
                        nc.vector.tensor_mul(ur[0:64, :], psrc[64:128, :], sin_sb[0:64, tsl])
                        nc.vector.tensor_mul(ur[64:128, :], psrc[0:64, :], sin_sb[64:128, tsl])
                        nc.vector.tensor_add(dst[:, tsl], cr[:], ur[:])
                    # v: plain copy to SBUF (fp32, feeds PE transpose)
                    vt_q = rope.tile([128, SW], F32, tag="vtq", name=f"vtq{i}")
                    nc.scalar.copy(vt_q[:], pv[:])
                    vt_tiles[i] = vt_q

                def emit_v_transpose(i):
                    # transpose V^T -> V for sub-quarter i (2 token chunks)
                    SW = TQ // 2
                    pvt = ps.tile([128, SW], F32, tag="ps", name=f"pvt{i}")
                    for j in range(2):
                        nc.tensor.transpose(
                            pvt[:, j * 128:(j + 1) * 128],
                            vt_tiles[i][:, j * 128:(j + 1) * 128],
                            id_sb[:])
                    nc.vector.tensor_copy(v_sb[:, i * SW:(i + 1) * SW], pvt[:])

                def emit_attn_head(tw0, twl, h):
                    wsl = slice(tw0, tw0 + twl)
                    nch = (tw0 + twl) // 128
                    w = tw0 // 128  # first diagonal chunk index
                    if True:
                        # phase 1: scores -> exp -> causal-zero, decoupled from pV
                        pts = []
                        for cc in range(nch):
                            # columns < rel are entirely above-diagonal for this
                            # chunk; skip them when a 256-wide tail still remains
                            rel = cc * 128 - tw0
                            n0 = 256 if (rel >= 256 and twl - 256 >= 256) else 0
                            csl = slice(n0, twl)
                            sc_ps = ps.tile([128, twl], F32, tag="ps",
                                            name=f"sc{w}_{h}_{cc}")
                            nc.tensor.matmul(
                                sc_ps[:, csl],
                                kt_sb[:, cc * 128:(cc + 1) * 128],
                                qt_sb[h][:, tw0 + n0:tw0 + twl], start=True, stop=True)
                            pt = ptp.tile([128, twl], F32R, tag="pt",
                                          name=f"pt{w}_{h}_{cc}")
                            nc.scalar.activation(pt[:, csl], sc_ps[:, csl], EXP,
                                                 scale=float(SCALE))
                            if cc >= w:
                                # zero strictly-above-diagonal: keep where tq >= tk
                                nc.gpsimd.affine_select(
                                    out=pt[:, csl], in_=pt[:, csl],
                                    pattern=[[1, twl - n0]],
                                    compare_op=mybir.AluOpType.is_ge, fill=0.0,
                                    base=tw0 + n0 - cc * 128, channel_multiplier=-1)
                            pts.append((pt, n0))
                        # phase 2: y^T += V^T-chunks @ probs, sums via ones-matmul
                        y_ps = ps.tile([128, twl], F32, tag="ps", name=f"y{w}_{h}")
                        s_ps = ps.tile([128, twl], F32, tag="ps", name=f"s{w}_{h}")
                        for cc in range(nch):
                            st = cc == 0
                            sp = cc == nch - 1
                            pt, n0 = pts[cc]
                            csl = slice(n0, twl)
                            nc.tensor.matmul(
                                y_ps[:, csl], v_sb[:, cc * DH:(cc + 1) * DH],
                                pt[:, csl], start=st, stop=sp)
                            nc.tensor.matmul(
                                s_ps[:, csl], ones_sb[:], pt[:, csl],
                                start=st, stop=sp)
                        rc = rcp.tile([128, twl], F32, tag="rc", name=f"rc{w}_{h}")
                        nc.vector.reciprocal(rc[:], s_ps[:])
                        nc.vector.tensor_mul(yt_sb[h][:, wsl], y_ps[:], rc[:])

                def emit_outproj_window(tw0, twl, split_out=False):
                    for j in range(twl // 128):
                        t0 = tw0 + j * 128
                        po = [ps.tile([128, 512], F32, tag="ps",
                                      name=f"po{t0}_{e}") for e in range(4)]
                        for k in range(HPD):
                            for e in range(4):
                                nc.tensor.matmul(
                                    po[e][:],
                                    yt_sb[k][:, t0:t0 + 128],
                                    wp_sb[:, k * C + e * 512:k * C + (e + 1) * 512],
                                    start=(k == 0), stop=(k == HPD - 1))
                        ost = ostp.tile([128, C], F32, tag="ost", name=f"ost{t0}")
                        if split_out:
                            for e in range(4):
                                esl = slice(e * 512, (e + 1) * 512)
                                nc.any.tensor_copy(ost[:, esl], po[e][:])
                                nc.sync.dma_start(out[t0:t0 + 128, esl], ost[:, esl])
                        else:
                            for e in range(4):
                                nc.any.tensor_copy(ost[:, e * 512:(e + 1) * 512], po[e][:])
                            nc.sync.dma_start(out[t0:t0 + 128, :], ost[:])

                with (
                    tc.tile_pool(name="pt", bufs=17) as ptp,
                    tc.tile_pool(name="rc", bufs=2) as rcp,
                    tc.tile_pool(name="ost", bufs=3) as ostp,
                ):
                    SW = TQ // 2
                    for qq in range(2 * NQ):
                        emit_xt_loads(qq)
                    for i in range(2 * NQ):
                        if i >= 1:
                            w0 = (i - 1) * SW

                            def mid(w0=w0, i=i):
                                emit_v_transpose(i - 1)
                                emit_attn_head(w0, SW, 0)
                        else:
                            mid = None
                        emit_proj_subq(i, mid_fn=mid, hook_ct=(2 if i <= 3 else 8))
                        if i >= 1:
                            emit_attn_head((i - 1) * SW, SW, 1)
                        if i >= 2:
                            emit_outproj_window((i - 2) * SW, SW)
                    emit_v_transpose(2 * NQ - 1)
                    emit_attn_head((2 * NQ - 1) * SW, SW, 0)
                    emit_outproj_window((2 * NQ - 2) * SW, SW, split_out=True)
                    emit_attn_head((2 * NQ - 1) * SW, SW, 1)
                    emit_outproj_window((2 * NQ - 1) * SW, SW, split_out=True)

    nc.compile()
    return nc


_CACHE = {}


def _get_module():
    if "nc" not in _CACHE:
        nc = bacc.Bacc("TRN2", target_bir_lowering=False, debug=False)
        _CACHE["nc"] = _emit(nc)
    return _CACHE["nc"]


def _host_constants():
    if "consts" in _CACHE:
        return _CACHE["consts"]
    inv_freq = 1.0 / (ROPE_BASE ** (np.arange(0, DH, 2, dtype=np.float64) / DH))
    ang = np.outer(np.arange(T, dtype=np.float64), inv_freq)      # (T, 64)
    emb = np.concatenate([ang, ang], axis=-1)                     # (T, 128)
    cos = np.cos(emb).astype(np.float32)                          # (T, 128)
    sin = np.sin(emb).astype(np.float32)
    cosT = np.ascontiguousarray(cos.T)                            # (128, T)
    sinT = np.ascontiguousarray(sin.T)
    sign = np.where(np.arange(DH) < DH // 2, -1.0, 1.0).astype(np.float32)
    sinR = np.ascontiguousarray(sinT * sign[:, None])
    ones = np.ones((128, 128), dtype=np.float32)
    ident = np.eye(128, dtype=np.float32)
    _CACHE["consts"] = (cosT, sinR, ones, ident)
    return _CACHE["consts"]


def kernel(x, wq, wk, wv, wproj):
    x = np.asarray(x, dtype=np.float32)
    wq = np.asarray(wq, dtype=np.float32)
    wk = np.asarray(wk, dtype=np.float32)
    wv = np.asarray(wv, dtype=np.float32)
    wproj = np.asarray(wproj, dtype=np.float32)

    nc = _get_module()
    cosT, sinR, ones, ident = _host_constants()
    xT = np.ascontiguousarray(x[0].T)                             # (C, T)

    in_maps = []
    for d in range(N_CORES):
        h0 = HPD * d
        g = d // 2
        # wq columns for heads h0..h0+HPD-1 -> [128, NCT*HPD*DH] (c-tile major)
        wq_d = wq[:, h0 * DH:(h0 + HPD) * DH]                     # (C, HPD*DH)
        wq_l = np.ascontiguousarray(
            wq_d.reshape(NCT, 128, HPD * DH).transpose(1, 0, 2).reshape(128, -1))
        wk_d = wk[:, g * DH:(g + 1) * DH]
        wk_l = np.ascontiguousarray(
            wk_d.reshape(NCT, 128, DH).transpose(1, 0, 2).reshape(128, -1))
        wv_d = wv[:, g * DH:(g + 1) * DH]
        wv_l = np.ascontiguousarray(
            wv_d.reshape(NCT, 128, DH).transpose(1, 0, 2).reshape(128, -1))
        # wproj rows for our heads -> [128, HPD*C] (head-major free dim)
        wp_d = wproj[h0 * DH:(h0 + HPD) * DH, :]                  # (HPD*DH, C)
        wp_l = np.ascontiguousarray(
            wp_d.reshape(HPD, 128, C).transpose(1, 0, 2).reshape(128, -1))
        in_maps.append({
            "xT": xT, "wq": wq_l, "wk": wk_l, "wv": wv_l, "wp": wp_l,
            "cosT": cosT, "sinR": sinR, "ones": ones, "ident": ident,
        })

    res = run_bass_kernel_spmd(nc, in_maps, core_ids=list(range(N_CORES)))
    acc = res.results[0]["out"].astype(np.float32)
    for d in range(1, N_CORES):
        acc = acc + res.results[d]["out"].astype(np.float32)
    return acc.reshape(1, T, C)


# revision 31
# speedup vs baseline: 1.1197x; 1.0333x over previous
"""Trainium2 Bass kernel for CausalSelfAttentionModern (GQA + RoPE + causal SDPA).

Sharding: tensor-parallel over heads across 8 NeuronCores.
Device d owns q-heads {2d, 2d+1} and kv-head d//2.
Each device computes its heads' attention plus its slice of the output
projection (row-parallel); the host sums the 8 partial outputs.

All matmuls run as float32r (full-rate fp32 mode on the PE array).
"""

import numpy as np
import concourse.bacc as bacc
import concourse.tile as tile
import concourse.mybir as mybir
from concourse.bass_utils import run_bass_kernel_spmd

F32 = mybir.dt.float32
F32R = mybir.dt.float32r
EXP = mybir.ActivationFunctionType.Exp

# hardcoded problem shapes
T = 2048          # sequence length
C = 2048          # embedding dim
DH = 128          # head dim
NH = 16           # query heads
NKV = 4           # kv heads
N_CORES = 8
HPD = NH // N_CORES  # q-heads per device = 2
ROPE_BASE = 10000.0
SCALE = 1.0 / np.sqrt(DH)

NQ = 4            # t-quarters for projection phase
TQ = T // NQ      # 512
NW = 4            # attention tq windows
TW = T // NW      # 512
NCT = C // 128    # 16 contraction tiles
NTC = T // 128    # 16 token chunks


def _emit(nc):
    xT = nc.dram_tensor("xT", [C, T], F32R, kind="ExternalInput").ap()
    wq = nc.dram_tensor("wq", [128, NCT * HPD * DH], F32R, kind="ExternalInput").ap()
    wk = nc.dram_tensor("wk", [128, NCT * DH], F32R, kind="ExternalInput").ap()
    wv = nc.dram_tensor("wv", [128, NCT * DH], F32R, kind="ExternalInput").ap()
    wp = nc.dram_tensor("wp", [128, HPD * C], F32R, kind="ExternalInput").ap()
    cosT = nc.dram_tensor("cosT", [128, T], F32, kind="ExternalInput").ap()
    sinR = nc.dram_tensor("sinR", [128, T], F32, kind="ExternalInput").ap()
    ones = nc.dram_tensor("ones", [128, 128], F32R, kind="ExternalInput").ap()
    ident = nc.dram_tensor("ident", [128, 128], F32, kind="ExternalInput").ap()
    out = nc.dram_tensor("out", [T, C], F32, kind="ExternalOutput").ap()

    with tile.TileContext(nc) as tc:
        with (
            tc.tile_pool(name="cst", bufs=1) as cst,
            tc.tile_pool(name="ps", bufs=8, space="PSUM") as ps,
        ):
            # persistent SBUF tensors (DMAs emitted at first-use points below)
            cos_sb = cst.tile([128, T], F32, tag="cos")
            sin_sb = cst.tile([128, T], F32, tag="sin")
            ones_sb = cst.tile([128, 128], F32R, tag="ones")
            id_sb = cst.tile([128, 128], F32, tag="ident")
            wp_sb = cst.tile([128, HPD * C], F32R, tag="wp")

            qt_sb = [cst.tile([128, T], F32R, tag=f"qt{m}", name=f"qt{m}")
                     for m in range(HPD)]
            kt_sb = cst.tile([128, T], F32R, tag="kt")
            vtp_pool = None  # vt quarter tiles come from the rope pool
            v_sb = cst.tile([128, NTC * DH], F32R, tag="v")
            yt_sb = [cst.tile([128, T], F32R, tag=f"yt{m}", name=f"yt{m}")
                     for m in range(HPD)]

            # ---------------- projections + rope, per t-quarter ----------------
            with (
                tc.tile_pool(name="wqkv", bufs=1) as wqkv,
                tc.tile_pool(name="xts", bufs=5) as xts,
                tc.tile_pool(name="rope", bufs=1) as rope,
            ):
                wq_sb = wqkv.tile([128, NCT * HPD * DH], F32R, tag="wq")
                wk_sb = wqkv.tile([128, NCT * DH], F32R, tag="wk")
                wv_sb = wqkv.tile([128, NCT * DH], F32R, tag="wv")
                # weights on the ACT ring: c-tile groups so sems fire early
                for a, b in [(0, 1), (1, 4), (4, 8), (8, 16)]:
                    q1 = HPD * DH
                    nc.scalar.dma_start(wq_sb[:, a * q1:b * q1], wq[:, a * q1:b * q1])
                    nc.scalar.dma_start(wk_sb[:, a * DH:b * DH], wk[:, a * DH:b * DH])
                    nc.scalar.dma_start(wv_sb[:, a * DH:b * DH], wv[:, a * DH:b * DH])

                xt_tiles = {}
                vt_tiles = {}

                def emit_xt_loads(qq):
                    # per half (8 c-tiles x 256 tokens) strided load
                    SW = TQ // 2
                    tsl = slice(qq * SW, (qq + 1) * SW)
                    for half in range(2):
                        xt = xts.tile([128, 8 * SW], F32R, tag="xt",
                                      name=f"xtq{qq}_{half}")
                        c0 = half * 8
                        splits = [(0, 4), (4, 8)] if (qq == 0 and half == 0) else [(0, 8)]
                        for a, b in splits:
                            nc.sync.dma_start(
                                xt[:, a * SW:b * SW].rearrange("p (ct t) -> p ct t", t=SW),
                                xT[(c0 + a) * 128:(c0 + b) * 128, tsl].rearrange(
                                    "(ct p) t -> p ct t", p=128))
                        xt_tiles[(qq, half)] = xt

                def emit_proj_subq(i, mid_fn=None, hook_ct=8):
                    # one 256-token sub-quarter: psum tiles complete before rope
                    SW = TQ // 2
                    tsl = slice(i * SW, (i + 1) * SW)
                    pq = [ps.tile([128, SW], F32, tag="ps", name=f"pq{i}_{m}")
                          for m in range(HPD)]
                    pk = ps.tile([128, SW], F32, tag="ps", name=f"pk{i}")
                    pv = ps.tile([128, SW], F32, tag="ps", name=f"pv{i}")
                    for ct in range(NCT):
                        if ct == hook_ct and mid_fn is not None:
                            mid_fn()
                        xt = xt_tiles[(i, ct // 8)]
                        xsl = slice((ct % 8) * SW, (ct % 8 + 1) * SW)
                        st = ct == 0
                        sp = ct == NCT - 1
                        for m in range(HPD):
                            nc.tensor.matmul(
                                pq[m][:],
                                wq_sb[:, (ct * HPD + m) * DH:(ct * HPD + m + 1) * DH],
                                xt[:, xsl], start=st, stop=sp)
                        nc.tensor.matmul(
                            pk[:], wk_sb[:, ct * DH:(ct + 1) * DH],
                            xt[:, xsl], start=st, stop=sp)
                        nc.tensor.matmul(
                            pv[:], wv_sb[:, ct * DH:(ct + 1) * DH],
                            xt[:, xsl], start=st, stop=sp)

                    if i == 0:
                        # constants needed from the rope/attention phases on
                        nc.scalar.dma_start(cos_sb[:], cosT[:])
                        nc.scalar.dma_start(sin_sb[:], sinR[:])
                        nc.scalar.dma_start(id_sb[:], ident[:])
                        nc.scalar.dma_start(ones_sb[:], ones[:])
                    if i == 1:
                        nc.scalar.dma_start(wp_sb[:], wp[:])

                    # rope: out = psum*cos + shift(psum)*sinR  (shift = rotate-half)
                    for psrc, dst in [(pq[0], qt_sb[0]), (pq[1], qt_sb[1]), (pk, kt_sb)]:
                        cr = rope.tile([128, SW], F32, tag="crope")
                        nc.vector.tensor_mul(cr[:], psrc[:], cos_sb[:, tsl])
                        ur = rope.tile([128, SW], F32, tag="urot")
                        nc.vector.tensor_mul(ur[0:64, :], psrc[64:128, :], sin_sb[0:64, tsl])
                        nc.vector.tensor_mul(ur[64:128, :], psrc[0:64, :], sin_sb[64:128, tsl])
                        nc.vector.tensor_add(dst[:, tsl], cr[:], ur[:])
                    # v: plain copy to SBUF (fp32, feeds PE transpose)
                    vt_q = rope.tile([128, SW], F32, tag="vtq", name=f"vtq{i}")
                    nc.scalar.copy(vt_q[:], pv[:])
                    vt_tiles[i] = vt_q

                def emit_v_transpose(i):
                    # transpose V^T -> V for sub-quarter i (2 token chunks)
                    SW = TQ // 2
                    pvt = ps.tile([128, SW], F32, tag="ps", name=f"pvt{i}")
                    for j in range(2):
                        nc.tensor.transpose(
                            pvt[:, j * 128:(j + 1) * 128],
                            vt_tiles[i][:, j * 128:(j + 1) * 128],
                            id_sb[:])
                    nc.vector.tensor_copy(v_sb[:, i * SW:(i + 1) * SW], pvt[:])

                def emit_attn_head(tw0, twl, h):
                    wsl = slice(tw0, tw0 + twl)
                    nch = (tw0 + twl) // 128
                    w = tw0 // 128  # first diagonal chunk index
                    if True:
                        # phase 1: scores -> exp -> causal-zero, decoupled from pV
                        # chunk PAIRS share one psum bank and one exp instruction
                        pts = []
                        for cp in range(0, nch, 2):
                            npair = min(2, nch - cp)
                            pw = npair * twl
                            sc_ps = ps.tile([128, pw], F32, tag="ps",
                                            name=f"sc{w}_{h}_{cp}")
                            for k2 in range(npair):
                                cc = cp + k2
                                nc.tensor.matmul(
                                    sc_ps[:, k2 * twl:(k2 + 1) * twl],
                                    kt_sb[:, cc * 128:(cc + 1) * 128],
                                    qt_sb[h][:, wsl], start=True, stop=True)
                            pt = ptp.tile([128, pw], F32R, tag="pt",
                                          name=f"pt{w}_{h}_{cp}")
                            nc.scalar.activation(pt[:], sc_ps[:], EXP,
                                                 scale=float(SCALE))
                            for k2 in range(npair):
                                cc = cp + k2
                                if cc >= w:
                                    # zero strictly-above-diagonal: keep tq >= tk
                                    nc.gpsimd.affine_select(
                                        out=pt[:, k2 * twl:(k2 + 1) * twl],
                                        in_=pt[:, k2 * twl:(k2 + 1) * twl],
                                        pattern=[[1, twl]],
                                        compare_op=mybir.AluOpType.is_ge, fill=0.0,
                                        base=tw0 - cc * 128, channel_multiplier=-1)
                                pts.append((pt, k2 * twl))
                        # phase 2: y^T += V^T-chunks @ probs, sums via ones-matmul
                        y_ps = ps.tile([128, twl], F32, tag="ps", name=f"y{w}_{h}")
                        s_ps = ps.tile([128, twl], F32, tag="ps", name=f"s{w}_{h}")
                        for cc in range(nch):
                            st = cc == 0
                            sp = cc == nch - 1
                            pt, off = pts[cc]
                            psl = slice(off, off + twl)
                            nc.tensor.matmul(
                                y_ps[:], v_sb[:, cc * DH:(cc + 1) * DH],
                                pt[:, psl], start=st, stop=sp)
                            nc.tensor.matmul(
                                s_ps[:], ones_sb[:], pt[:, psl],
                                start=st, stop=sp)
                        rc = rcp.tile([128, twl], F32, tag="rc", name=f"rc{w}_{h}")
                        nc.vector.reciprocal(rc[:], s_ps[:])
                        nc.vector.tensor_mul(yt_sb[h][:, wsl], y_ps[:], rc[:])

                def emit_outproj_window(tw0, twl, split_out=False):
                    for j in range(twl // 128):
                        t0 = tw0 + j * 128
                        po = [ps.tile([128, 512], F32, tag="ps",
                                      name=f"po{t0}_{e}") for e in range(4)]
                        for k in range(HPD):
                            for e in range(4):
                                nc.tensor.matmul(
                                    po[e][:],
                                    yt_sb[k][:, t0:t0 + 128],
                                    wp_sb[:, k * C + e * 512:k * C + (e + 1) * 512],
                                    start=(k == 0), stop=(k == HPD - 1))
                        ost = ostp.tile([128, C], F32, tag="ost", name=f"ost{t0}")
                        if split_out:
                            for e in range(4):
                                esl = slice(e * 512, (e + 1) * 512)
                                nc.any.tensor_copy(ost[:, esl], po[e][:])
                                nc.sync.dma_start(out[t0:t0 + 128, esl], ost[:, esl])
                        else:
                            for e in range(4):
                                nc.any.tensor_copy(ost[:, e * 512:(e + 1) * 512], po[e][:])
                            nc.sync.dma_start(out[t0:t0 + 128, :], ost[:])

                with (
                    tc.tile_pool(name="pt", bufs=10) as ptp,
                    tc.tile_pool(name="rc", bufs=2) as rcp,
                    tc.tile_pool(name="ost", bufs=3) as ostp,
                ):
                    SW = TQ // 2
                    for qq in range(2 * NQ):
                        emit_xt_loads(qq)
                    for i in range(2 * NQ):
                        if i >= 1:
                            w0 = (i - 1) * SW

                            def mid(w0=w0, i=i):
                                emit_v_transpose(i - 1)
                                emit_attn_head(w0, SW, 0)
                        else:
                            mid = None
                        emit_proj_subq(i, mid_fn=mid, hook_ct=(2 if i <= 3 else 8))
                        if i >= 1:
                            emit_attn_head((i - 1) * SW, SW, 1)
                        if i >= 2:
                            emit_outproj_window((i - 2) * SW, SW)
                    emit_v_transpose(2 * NQ - 1)
                    emit_attn_head((2 * NQ - 1) * SW, SW, 0)
                    emit_outproj_window((2 * NQ - 2) * SW, SW, split_out=True)
                    emit_attn_head((2 * NQ - 1) * SW, SW, 1)
                    emit_outproj_window((2 * NQ - 1) * SW, SW, split_out=True)

    nc.compile()
    return nc


_CACHE = {}


def _get_module():
    if "nc" not in _CACHE:
        nc = bacc.Bacc("TRN2", target_bir_lowering=False, debug=False)
        _CACHE["nc"] = _emit(nc)
    return _CACHE["nc"]


def _host_constants():
    if "consts" in _CACHE:
        return _CACHE["consts"]
    inv_freq = 1.0 / (ROPE_BASE ** (np.arange(0, DH, 2, dtype=np.float64) / DH))
    ang = np.outer(np.arange(T, dtype=np.float64), inv_freq)      # (T, 64)
    emb = np.concatenate([ang, ang], axis=-1)                     # (T, 128)
    cos = np.cos(emb).astype(np.float32)                          # (T, 128)
    sin = np.sin(emb).astype(np.float32)
    cosT = np.ascontiguousarray(cos.T)                            # (128, T)
    sinT = np.ascontiguousarray(sin.T)
    sign = np.where(np.arange(DH) < DH // 2, -1.0, 1.0).astype(np.float32)
    sinR = np.ascontiguousarray(sinT * sign[:, None])
    ones = np.ones((128, 128), dtype=np.float32)
    ident = np.eye(128, dtype=np.float32)
    _CACHE["consts"] = (cosT, sinR, ones, ident)
    return _CACHE["consts"]


def kernel(x, wq, wk, wv, wproj):
    x = np.asarray(x, dtype=np.float32)
    wq = np.asarray(wq, dtype=np.float32)
    wk = np.asarray(wk, dtype=np.float32)
    wv = np.asarray(wv, dtype=np.float32)
    wproj = np.asarray(wproj, dtype=np.float32)

    nc = _get_module()
    cosT, sinR, ones, ident = _host_constants()
    xT = np.ascontiguousarray(x[0].T)                             # (C, T)

    in_maps = []
    for d in range(N_CORES):
        h0 = HPD * d
        g = d // 2
        # wq columns for heads h0..h0+HPD-1 -> [128, NCT*HPD*DH] (c-tile major)
        wq_d = wq[:, h0 * DH:(h0 + HPD) * DH]                     # (C, HPD*DH)
        wq_l = np.ascontiguousarray(
            wq_d.reshape(NCT, 128, HPD * DH).transpose(1, 0, 2).reshape(128, -1))
        wk_d = wk[:, g * DH:(g + 1) * DH]
        wk_l = np.ascontiguousarray(
            wk_d.reshape(NCT, 128, DH).transpose(1, 0, 2).reshape(128, -1))
        wv_d = wv[:, g * DH:(g + 1) * DH]
        wv_l = np.ascontiguousarray(
            wv_d.reshape(NCT, 128, DH).transpose(1, 0, 2).reshape(128, -1))
        # wproj rows for our heads -> [128, HPD*C] (head-major free dim)
        wp_d = wproj[h0 * DH:(h0 + HPD) * DH, :]                  # (HPD*DH, C)
        wp_l = np.ascontiguousarray(
            wp_d.reshape(HPD, 128, C).transpose(1, 0, 2).reshape(128, -1))
        in_maps.append({
            "xT": xT, "wq": wq_l, "wk": wk_l, "wv": wv_l, "wp": wp_l,
            "cosT": cosT, "sinR": sinR, "ones": ones, "ident": ident,
        })

    res = run_bass_kernel_spmd(nc, in_maps, core_ids=list(range(N_CORES)))
    acc = res.results[0]["out"].astype(np.float32)
    for d in range(1, N_CORES):
        acc = acc + res.results[d]["out"].astype(np.float32)
    return acc.reshape(1, T, C)


# revision 35
# speedup vs baseline: 1.1339x; 1.0127x over previous
"""Trainium2 Bass kernel for CausalSelfAttentionModern (GQA + RoPE + causal SDPA).

Sharding: tensor-parallel over heads across 8 NeuronCores.
Device d owns q-heads {2d, 2d+1} and kv-head d//2.
Each device computes its heads' attention plus its slice of the output
projection (row-parallel); the host sums the 8 partial outputs.

All matmuls run as float32r (full-rate fp32 mode on the PE array).
"""

import numpy as np
import concourse.bacc as bacc
import concourse.tile as tile
import concourse.mybir as mybir
from concourse.bass_utils import run_bass_kernel_spmd

F32 = mybir.dt.float32
F32R = mybir.dt.float32r
EXP = mybir.ActivationFunctionType.Exp

# hardcoded problem shapes
T = 2048          # sequence length
C = 2048          # embedding dim
DH = 128          # head dim
NH = 16           # query heads
NKV = 4           # kv heads
N_CORES = 8
HPD = NH // N_CORES  # q-heads per device = 2
ROPE_BASE = 10000.0
SCALE = 1.0 / np.sqrt(DH)

NQ = 4            # t-quarters for projection phase
TQ = T // NQ      # 512
NW = 4            # attention tq windows
TW = T // NW      # 512
NCT = C // 128    # 16 contraction tiles
NTC = T // 128    # 16 token chunks


def _emit(nc):
    xT = nc.dram_tensor("xT", [C, T], F32R, kind="ExternalInput").ap()
    wq = nc.dram_tensor("wq", [128, NCT * HPD * DH], F32R, kind="ExternalInput").ap()
    wk = nc.dram_tensor("wk", [128, NCT * DH], F32R, kind="ExternalInput").ap()
    wv = nc.dram_tensor("wv", [128, NCT * DH], F32R, kind="ExternalInput").ap()
    wp = nc.dram_tensor("wp", [128, HPD * C], F32R, kind="ExternalInput").ap()
    cosT = nc.dram_tensor("cosT", [128, T], F32, kind="ExternalInput").ap()
    sinR = nc.dram_tensor("sinR", [128, T], F32, kind="ExternalInput").ap()
    ones = nc.dram_tensor("ones", [128, 128], F32R, kind="ExternalInput").ap()
    ident = nc.dram_tensor("ident", [128, 128], F32, kind="ExternalInput").ap()
    out = nc.dram_tensor("out", [T, C], F32, kind="ExternalOutput").ap()

    with tile.TileContext(nc) as tc:
        with (
            tc.tile_pool(name="cst", bufs=1) as cst,
            tc.tile_pool(name="ps", bufs=8, space="PSUM") as ps,
        ):
            # persistent SBUF tensors (DMAs emitted at first-use points below)
            cos_sb = cst.tile([128, T], F32, tag="cos")
            sin_sb = cst.tile([128, T], F32, tag="sin")
            ones_sb = cst.tile([128, 128], F32R, tag="ones")
            id_sb = cst.tile([128, 128], F32, tag="ident")
            wp_sb = cst.tile([128, HPD * C], F32R, tag="wp")

            qt_sb = [cst.tile([128, T], F32R, tag=f"qt{m}", name=f"qt{m}")
                     for m in range(HPD)]
            kt_sb = cst.tile([128, T], F32R, tag="kt")
            vtp_pool = None  # vt quarter tiles come from the rope pool
            v_sb = cst.tile([128, NTC * DH], F32R, tag="v")
            yt_sb = [cst.tile([128, T], F32R, tag=f"yt{m}", name=f"yt{m}")
                     for m in range(HPD)]

            # ---------------- projections + rope, per t-quarter ----------------
            with (
                tc.tile_pool(name="wqkv", bufs=1) as wqkv,
                tc.tile_pool(name="xts", bufs=5) as xts,
                tc.tile_pool(name="rope", bufs=1) as rope,
            ):
                wq_sb = wqkv.tile([128, NCT * HPD * DH], F32R, tag="wq")
                wk_sb = wqkv.tile([128, NCT * DH], F32R, tag="wk")
                wv_sb = wqkv.tile([128, NCT * DH], F32R, tag="wv")
                # weights on the ACT ring: c-tile groups so sems fire early
                for a, b in [(0, 1), (1, 2), (2, 4), (4, 8), (8, 16)]:
                    q1 = HPD * DH
                    nc.scalar.dma_start(wq_sb[:, a * q1:b * q1], wq[:, a * q1:b * q1])
                    nc.scalar.dma_start(wk_sb[:, a * DH:b * DH], wk[:, a * DH:b * DH])
                    nc.scalar.dma_start(wv_sb[:, a * DH:b * DH], wv[:, a * DH:b * DH])

                xt_tiles = {}
                vt_tiles = {}

                def emit_xt_loads(qq):
                    # per half (8 c-tiles x 256 tokens) strided load
                    SW = TQ // 2
                    tsl = slice(qq * SW, (qq + 1) * SW)
                    for half in range(2):
                        xt = xts.tile([128, 8 * SW], F32R, tag="xt",
                                      name=f"xtq{qq}_{half}")
                        c0 = half * 8
                        splits = [(0, 2), (2, 4), (4, 8)] if (qq == 0 and half == 0) else [(0, 8)]
                        for a, b in splits:
                            nc.sync.dma_start(
                                xt[:, a * SW:b * SW].rearrange("p (ct t) -> p ct t", t=SW),
                                xT[(c0 + a) * 128:(c0 + b) * 128, tsl].rearrange(
                                    "(ct p) t -> p ct t", p=128))
                        xt_tiles[(qq, half)] = xt

                def emit_proj_subq(i, mid_fn=None, hook_ct=8):
                    # one 256-token sub-quarter: psum tiles complete before rope
                    SW = TQ // 2
                    tsl = slice(i * SW, (i + 1) * SW)
                    pq = [ps.tile([128, SW], F32, tag="ps", name=f"pq{i}_{m}")
                          for m in range(HPD)]
                    pk = ps.tile([128, SW], F32, tag="ps", name=f"pk{i}")
                    pv = ps.tile([128, SW], F32, tag="ps", name=f"pv{i}")
                    for ct in range(NCT):
                        if ct == hook_ct and mid_fn is not None:
                            mid_fn()
                        xt = xt_tiles[(i, ct // 8)]
                        xsl = slice((ct % 8) * SW, (ct % 8 + 1) * SW)
                        st = ct == 0
                        sp = ct == NCT - 1
                        for m in range(HPD):
                            nc.tensor.matmul(
                                pq[m][:],
                                wq_sb[:, (ct * HPD + m) * DH:(ct * HPD + m + 1) * DH],
                                xt[:, xsl], start=st, stop=sp)
                        nc.tensor.matmul(
                            pk[:], wk_sb[:, ct * DH:(ct + 1) * DH],
                            xt[:, xsl], start=st, stop=sp)
                        nc.tensor.matmul(
                            pv[:], wv_sb[:, ct * DH:(ct + 1) * DH],
                            xt[:, xsl], start=st, stop=sp)

                    if i == 0:
                        # constants needed from the rope/attention phases on
                        nc.scalar.dma_start(cos_sb[:], cosT[:])
                        nc.scalar.dma_start(sin_sb[:], sinR[:])
                        nc.scalar.dma_start(id_sb[:], ident[:])
                        nc.scalar.dma_start(ones_sb[:], ones[:])
                    if i == 1:
                        nc.scalar.dma_start(wp_sb[:], wp[:])

                    # rope: out = psum*cos + shift(psum)*sinR  (shift = rotate-half)
                    for psrc, dst in [(pq[0], qt_sb[0]), (pq[1], qt_sb[1]), (pk, kt_sb)]:
                        cr = rope.tile([128, SW], F32, tag="crope")
                        nc.vector.tensor_mul(cr[:], psrc[:], cos_sb[:, tsl])
                        ur = rope.tile([128, SW], F32, tag="urot")
                        nc.vector.tensor_mul(ur[0:64, :], psrc[64:128, :], sin_sb[0:64, tsl])
                        nc.vector.tensor_mul(ur[64:128, :], psrc[0:64, :], sin_sb[64:128, tsl])
                        nc.vector.tensor_add(dst[:, tsl], cr[:], ur[:])
                    # v: plain copy to SBUF (fp32, feeds PE transpose)
                    vt_q = rope.tile([128, SW], F32, tag="vtq", name=f"vtq{i}")
                    nc.scalar.copy(vt_q[:], pv[:])
                    vt_tiles[i] = vt_q

                def emit_v_transpose(i):
                    # transpose V^T -> V for sub-quarter i (2 token chunks)
                    SW = TQ // 2
                    pvt = ps.tile([128, SW], F32, tag="ps", name=f"pvt{i}")
                    for j in range(2):
                        nc.tensor.transpose(
                            pvt[:, j * 128:(j + 1) * 128],
                            vt_tiles[i][:, j * 128:(j + 1) * 128],
                            id_sb[:])
                    nc.vector.tensor_copy(v_sb[:, i * SW:(i + 1) * SW], pvt[:])

                def emit_attn_head(tw0, twl, h):
                    wsl = slice(tw0, tw0 + twl)
                    nch = (tw0 + twl) // 128
                    w = tw0 // 128  # first diagonal chunk index
                    if True:
                        # phase 1: scores -> exp -> causal-zero, decoupled from pV
                        # chunk PAIRS share one psum bank and one exp instruction
                        pts = []
                        for cp in range(0, nch, 2):
                            npair = min(2, nch - cp)
                            pw = npair * twl
                            sc_ps = ps.tile([128, pw], F32, tag="ps",
                                            name=f"sc{w}_{h}_{cp}")
                            for k2 in range(npair):
                                cc = cp + k2
                                nc.tensor.matmul(
                                    sc_ps[:, k2 * twl:(k2 + 1) * twl],
                                    kt_sb[:, cc * 128:(cc + 1) * 128],
                                    qt_sb[h][:, wsl], start=True, stop=True)
                            pt = ptp.tile([128, pw], F32R, tag="pt",
                                          name=f"pt{w}_{h}_{cp}")
                            nc.scalar.activation(pt[:], sc_ps[:], EXP,
                                                 scale=float(SCALE))
                            for k2 in range(npair):
                                cc = cp + k2
                                if cc >= w:
                                    # zero strictly-above-diagonal: keep tq >= tk
                                    nc.gpsimd.affine_select(
                                        out=pt[:, k2 * twl:(k2 + 1) * twl],
                                        in_=pt[:, k2 * twl:(k2 + 1) * twl],
                                        pattern=[[1, twl]],
                                        compare_op=mybir.AluOpType.is_ge, fill=0.0,
                                        base=tw0 - cc * 128, channel_multiplier=-1)
                                pts.append((pt, k2 * twl))
                        # phase 2: y^T += V^T-chunks @ probs, sums via ones-matmul
                        y_ps = ps.tile([128, twl], F32, tag="ps", name=f"y{w}_{h}")
                        s_ps = ps.tile([128, twl], F32, tag="ps", name=f"s{w}_{h}")
                        for cc in range(nch):
                            st = cc == 0
                            sp = cc == nch - 1
                            pt, off = pts[cc]
                            psl = slice(off, off + twl)
                            nc.tensor.matmul(
                                y_ps[:], v_sb[:, cc * DH:(cc + 1) * DH],
                                pt[:, psl], start=st, stop=sp)
                            nc.tensor.matmul(
                                s_ps[:], ones_sb[:], pt[:, psl],
                                start=st, stop=sp)
                        rc = rcp.tile([128, twl], F32, tag="rc", name=f"rc{w}_{h}")
                        nc.vector.reciprocal(rc[:], s_ps[:])
                        nc.vector.tensor_mul(yt_sb[h][:, wsl], y_ps[:], rc[:])

                def emit_outproj_window(tw0, twl, split_out=False):
                    for j in range(twl // 128):
                        t0 = tw0 + j * 128
                        po = [ps.tile([128, 512], F32, tag="ps",
                                      name=f"po{t0}_{e}") for e in range(4)]
                        for k in range(HPD):
                            for e in range(4):
                                nc.tensor.matmul(
                                    po[e][:],
                                    yt_sb[k][:, t0:t0 + 128],
                                    wp_sb[:, k * C + e * 512:k * C + (e + 1) * 512],
                                    start=(k == 0), stop=(k == HPD - 1))
                        ost = ostp.tile([128, C], F32, tag="ost", name=f"ost{t0}")
                        if split_out:
                            for e in range(4):
                                esl = slice(e * 512, (e + 1) * 512)
                                nc.any.tensor_copy(ost[:, esl], po[e][:])
                                nc.sync.dma_start(out[t0:t0 + 128, esl], ost[:, esl])
                        else:
                            for e in range(4):
                                nc.any.tensor_copy(ost[:, e * 512:(e + 1) * 512], po[e][:])
                            nc.sync.dma_start(out[t0:t0 + 128, :], ost[:])

                with (
                    tc.tile_pool(name="pt", bufs=10) as ptp,
                    tc.tile_pool(name="rc", bufs=2) as rcp,
                    tc.tile_pool(name="ost", bufs=3) as ostp,
                ):
                    SW = TQ // 2
                    for qq in range(2 * NQ):
                        emit_xt_loads(qq)
                    for i in range(2 * NQ):
                        if i >= 1:
                            w0 = (i - 1) * SW

                            def mid(w0=w0, i=i):
                                emit_v_transpose(i - 1)
                                emit_attn_head(w0, SW, 0)
                        else:
                            mid = None
                        emit_proj_subq(i, mid_fn=mid, hook_ct=(2 if i <= 3 else 8))
                        if i >= 1:
                            emit_attn_head((i - 1) * SW, SW, 1)
                        if i >= 2:
                            emit_outproj_window((i - 2) * SW, SW)
                    emit_v_transpose(2 * NQ - 1)
                    emit_attn_head((2 * NQ - 1) * SW, SW, 0)
                    emit_outproj_window((2 * NQ - 2) * SW, SW, split_out=True)
                    emit_attn_head((2 * NQ - 1) * SW, SW, 1)
                    emit_outproj_window((2 * NQ - 1) * SW, SW, split_out=True)

    nc.compile()
    return nc


_CACHE = {}


def _get_module():
    if "nc" not in _CACHE:
        nc = bacc.Bacc("TRN2", target_bir_lowering=False, debug=False)
        _CACHE["nc"] = _emit(nc)
    return _CACHE["nc"]


def _host_constants():
    if "consts" in _CACHE:
        return _CACHE["consts"]
    inv_freq = 1.0 / (ROPE_BASE ** (np.arange(0, DH, 2, dtype=np.float64) / DH))
    ang = np.outer(np.arange(T, dtype=np.float64), inv_freq)      # (T, 64)
    emb = np.concatenate([ang, ang], axis=-1)                     # (T, 128)
    cos = np.cos(emb).astype(np.float32)                          # (T, 128)
    sin = np.sin(emb).astype(np.float32)
    cosT = np.ascontiguousarray(cos.T)                            # (128, T)
    sinT = np.ascontiguousarray(sin.T)
    sign = np.where(np.arange(DH) < DH // 2, -1.0, 1.0).astype(np.float32)
    sinR = np.ascontiguousarray(sinT * sign[:, None])
    ones = np.ones((128, 128), dtype=np.float32)
    ident = np.eye(128, dtype=np.float32)
    _CACHE["consts"] = (cosT, sinR, ones, ident)
    return _CACHE["consts"]


def kernel(x, wq, wk, wv, wproj):
    x = np.asarray(x, dtype=np.float32)
    wq = np.asarray(wq, dtype=np.float32)
    wk = np.asarray(wk, dtype=np.float32)
    wv = np.asarray(wv, dtype=np.float32)
    wproj = np.asarray(wproj, dtype=np.float32)

    nc = _get_module()
    cosT, sinR, ones, ident = _host_constants()
    xT = np.ascontiguousarray(x[0].T)                             # (C, T)

    in_maps = []
    for d in range(N_CORES):
        h0 = HPD * d
        g = d // 2
        # wq columns for heads h0..h0+HPD-1 -> [128, NCT*HPD*DH] (c-tile major)
        wq_d = wq[:, h0 * DH:(h0 + HPD) * DH]                     # (C, HPD*DH)
        wq_l = np.ascontiguousarray(
            wq_d.reshape(NCT, 128, HPD * DH).transpose(1, 0, 2).reshape(128, -1))
        wk_d = wk[:, g * DH:(g + 1) * DH]
        wk_l = np.ascontiguousarray(
            wk_d.reshape(NCT, 128, DH).transpose(1, 0, 2).reshape(128, -1))
        wv_d = wv[:, g * DH:(g + 1) * DH]
        wv_l = np.ascontiguousarray(
            wv_d.reshape(NCT, 128, DH).transpose(1, 0, 2).reshape(128, -1))
        # wproj rows for our heads -> [128, HPD*C] (head-major free dim)
        wp_d = wproj[h0 * DH:(h0 + HPD) * DH, :]                  # (HPD*DH, C)
        wp_l = np.ascontiguousarray(
            wp_d.reshape(HPD, 128, C).transpose(1, 0, 2).reshape(128, -1))
        in_maps.append({
            "xT": xT, "wq": wq_l, "wk": wk_l, "wv": wv_l, "wp": wp_l,
            "cosT": cosT, "sinR": sinR, "ones": ones, "ident": ident,
        })

    res = run_bass_kernel_spmd(nc, in_maps, core_ids=list(range(N_CORES)))
    acc = res.results[0]["out"].astype(np.float32)
    for d in range(1, N_CORES):
        acc = acc + res.results[d]["out"].astype(np.float32)
    return acc.reshape(1, T, C)


# revision 36
# speedup vs baseline: 1.1426x; 1.0076x over previous
"""Trainium2 Bass kernel for CausalSelfAttentionModern (GQA + RoPE + causal SDPA).

Sharding: tensor-parallel over heads across 8 NeuronCores.
Device d owns q-heads {2d, 2d+1} and kv-head d//2.
Each device computes its heads' attention plus its slice of the output
projection (row-parallel); the host sums the 8 partial outputs.

All matmuls run as float32r (full-rate fp32 mode on the PE array).
"""

import numpy as np
import concourse.bacc as bacc
import concourse.tile as tile
import concourse.mybir as mybir
from concourse.bass_utils import run_bass_kernel_spmd

F32 = mybir.dt.float32
F32R = mybir.dt.float32r
EXP = mybir.ActivationFunctionType.Exp

# hardcoded problem shapes
T = 2048          # sequence length
C = 2048          # embedding dim
DH = 128          # head dim
NH = 16           # query heads
NKV = 4           # kv heads
N_CORES = 8
HPD = NH // N_CORES  # q-heads per device = 2
ROPE_BASE = 10000.0
SCALE = 1.0 / np.sqrt(DH)

NQ = 4            # t-quarters for projection phase
TQ = T // NQ      # 512
NW = 4            # attention tq windows
TW = T // NW      # 512
NCT = C // 128    # 16 contraction tiles
NTC = T // 128    # 16 token chunks


def _emit(nc):
    xT = nc.dram_tensor("xT", [C, T], F32R, kind="ExternalInput").ap()
    wq = nc.dram_tensor("wq", [128, NCT * HPD * DH], F32R, kind="ExternalInput").ap()
    wk = nc.dram_tensor("wk", [128, NCT * DH], F32R, kind="ExternalInput").ap()
    wv = nc.dram_tensor("wv", [128, NCT * DH], F32R, kind="ExternalInput").ap()
    wp = nc.dram_tensor("wp", [128, HPD * C], F32R, kind="ExternalInput").ap()
    cosT = nc.dram_tensor("cosT", [128, T], F32, kind="ExternalInput").ap()
    sinR = nc.dram_tensor("sinR", [128, T], F32, kind="ExternalInput").ap()
    ones = nc.dram_tensor("ones", [128, 128], F32R, kind="ExternalInput").ap()
    ident = nc.dram_tensor("ident", [128, 128], F32, kind="ExternalInput").ap()
    out = nc.dram_tensor("out", [T, C], F32, kind="ExternalOutput").ap()

    with tile.TileContext(nc) as tc:
        with (
            tc.tile_pool(name="cst", bufs=1) as cst,
            tc.tile_pool(name="ps", bufs=8, space="PSUM") as ps,
        ):
            # persistent SBUF tensors (DMAs emitted at first-use points below)
            cos_sb = cst.tile([128, T], F32, tag="cos")
            sin_sb = cst.tile([128, T], F32, tag="sin")
            ones_sb = cst.tile([128, 128], F32R, tag="ones")
            id_sb = cst.tile([128, 128], F32, tag="ident")
            wp_sb = cst.tile([128, HPD * C], F32R, tag="wp")

            qt_sb = [cst.tile([128, T], F32R, tag=f"qt{m}", name=f"qt{m}")
                     for m in range(HPD)]
            kt_sb = cst.tile([128, T], F32R, tag="kt")
            vtp_pool = None  # vt quarter tiles come from the rope pool
            v_sb = cst.tile([128, NTC * DH], F32R, tag="v")
            yt_sb = [cst.tile([128, T], F32R, tag=f"yt{m}", name=f"yt{m}")
                     for m in range(HPD)]

            # ---------------- projections + rope, per t-quarter ----------------
            with (
                tc.tile_pool(name="wqkv", bufs=1) as wqkv,
                tc.tile_pool(name="xts", bufs=5) as xts,
                tc.tile_pool(name="rope", bufs=1) as rope,
            ):
                wq_sb = wqkv.tile([128, NCT * HPD * DH], F32R, tag="wq")
                wk_sb = wqkv.tile([128, NCT * DH], F32R, tag="wk")
                wv_sb = wqkv.tile([128, NCT * DH], F32R, tag="wv")
                # weights on the ACT ring: c-tile groups so sems fire early
                for a, b in [(0, 1), (1, 2), (2, 4), (4, 8), (8, 16)]:
                    q1 = HPD * DH
                    nc.scalar.dma_start(wq_sb[:, a * q1:b * q1], wq[:, a * q1:b * q1])
                    nc.scalar.dma_start(wk_sb[:, a * DH:b * DH], wk[:, a * DH:b * DH])
                    nc.scalar.dma_start(wv_sb[:, a * DH:b * DH], wv[:, a * DH:b * DH])

                xt_tiles = {}
                vt_tiles = {}

                def emit_xt_loads(qq):
                    # per half (8 c-tiles x 256 tokens) strided load
                    SW = TQ // 2
                    tsl = slice(qq * SW, (qq + 1) * SW)
                    for half in range(2):
                        xt = xts.tile([128, 8 * SW], F32R, tag="xt",
                                      name=f"xtq{qq}_{half}")
                        c0 = half * 8
                        splits = [(0, 2), (2, 4), (4, 8)] if (qq == 0 and half == 0) else [(0, 8)]
                        for a, b in splits:
                            nc.sync.dma_start(
                                xt[:, a * SW:b * SW].rearrange("p (ct t) -> p ct t", t=SW),
                                xT[(c0 + a) * 128:(c0 + b) * 128, tsl].rearrange(
                                    "(ct p) t -> p ct t", p=128))
                        xt_tiles[(qq, half)] = xt

                def emit_proj_subq(i, hooks=()):
                    # one 256-token sub-quarter: psum tiles complete before rope
                    SW = TQ // 2
                    tsl = slice(i * SW, (i + 1) * SW)
                    pq = [ps.tile([128, SW], F32, tag="ps", name=f"pq{i}_{m}")
                          for m in range(HPD)]
                    pk = ps.tile([128, SW], F32, tag="ps", name=f"pk{i}")
                    pv = ps.tile([128, SW], F32, tag="ps", name=f"pv{i}")
                    hooks = dict(hooks)
                    for ct in range(NCT):
                        fn = hooks.pop(ct, None)
                        if fn is not None:
                            fn()
                        xt = xt_tiles[(i, ct // 8)]
                        xsl = slice((ct % 8) * SW, (ct % 8 + 1) * SW)
                        st = ct == 0
                        sp = ct == NCT - 1
                        for m in range(HPD):
                            nc.tensor.matmul(
                                pq[m][:],
                                wq_sb[:, (ct * HPD + m) * DH:(ct * HPD + m + 1) * DH],
                                xt[:, xsl], start=st, stop=sp)
                        nc.tensor.matmul(
                            pk[:], wk_sb[:, ct * DH:(ct + 1) * DH],
                            xt[:, xsl], start=st, stop=sp)
                        nc.tensor.matmul(
                            pv[:], wv_sb[:, ct * DH:(ct + 1) * DH],
                            xt[:, xsl], start=st, stop=sp)

                    if i == 0:
                        # constants needed from the rope/attention phases on
                        nc.scalar.dma_start(cos_sb[:], cosT[:])
                        nc.scalar.dma_start(sin_sb[:], sinR[:])
                        nc.scalar.dma_start(id_sb[:], ident[:])
                        nc.scalar.dma_start(ones_sb[:], ones[:])
                    if i == 1:
                        nc.scalar.dma_start(wp_sb[:], wp[:])

                    # rope: out = psum*cos + shift(psum)*sinR  (shift = rotate-half)
                    for psrc, dst in [(pq[0], qt_sb[0]), (pq[1], qt_sb[1]), (pk, kt_sb)]:
                        cr = rope.tile([128, SW], F32, tag="crope")
                        nc.vector.tensor_mul(cr[:], psrc[:], cos_sb[:, tsl])
                        ur = rope.tile([128, SW], F32, tag="urot")
                        nc.vector.tensor_mul(ur[0:64, :], psrc[64:128, :], sin_sb[0:64, tsl])
                        nc.vector.tensor_mul(ur[64:128, :], psrc[0:64, :], sin_sb[64:128, tsl])
                        nc.vector.tensor_add(dst[:, tsl], cr[:], ur[:])
                    # v: plain copy to SBUF (fp32, feeds PE transpose)
                    vt_q = rope.tile([128, SW], F32, tag="vtq", name=f"vtq{i}")
                    nc.scalar.copy(vt_q[:], pv[:])
                    vt_tiles[i] = vt_q

                def emit_v_transpose(i):
                    # transpose V^T -> V for sub-quarter i (2 token chunks)
                    SW = TQ // 2
                    pvt = ps.tile([128, SW], F32, tag="ps", name=f"pvt{i}")
                    for j in range(2):
                        nc.tensor.transpose(
                            pvt[:, j * 128:(j + 1) * 128],
                            vt_tiles[i][:, j * 128:(j + 1) * 128],
                            id_sb[:])
                    nc.vector.tensor_copy(v_sb[:, i * SW:(i + 1) * SW], pvt[:])

                def emit_attn_head(tw0, twl, h):
                    wsl = slice(tw0, tw0 + twl)
                    nch = (tw0 + twl) // 128
                    w = tw0 // 128  # first diagonal chunk index
                    if True:
                        # phase 1: scores -> exp -> causal-zero, decoupled from pV
                        # chunk PAIRS share one psum bank and one exp instruction
                        pts = []
                        for cp in range(0, nch, 2):
                            npair = min(2, nch - cp)
                            pw = npair * twl
                            sc_ps = ps.tile([128, pw], F32, tag="ps",
                                            name=f"sc{w}_{h}_{cp}")
                            for k2 in range(npair):
                                cc = cp + k2
                                nc.tensor.matmul(
                                    sc_ps[:, k2 * twl:(k2 + 1) * twl],
                                    kt_sb[:, cc * 128:(cc + 1) * 128],
                                    qt_sb[h][:, wsl], start=True, stop=True)
                            pt = ptp.tile([128, pw], F32R, tag="pt",
                                          name=f"pt{w}_{h}_{cp}")
                            nc.scalar.activation(pt[:], sc_ps[:], EXP,
                                                 scale=float(SCALE))
                            for k2 in range(npair):
                                cc = cp + k2
                                if cc >= w:
                                    # zero strictly-above-diagonal: keep tq >= tk
                                    nc.gpsimd.affine_select(
                                        out=pt[:, k2 * twl:(k2 + 1) * twl],
                                        in_=pt[:, k2 * twl:(k2 + 1) * twl],
                                        pattern=[[1, twl]],
                                        compare_op=mybir.AluOpType.is_ge, fill=0.0,
                                        base=tw0 - cc * 128, channel_multiplier=-1)
                                pts.append((pt, k2 * twl))
                        # phase 2: y^T += V^T-chunks @ probs, sums via ones-matmul
                        y_ps = ps.tile([128, twl], F32, tag="ps", name=f"y{w}_{h}")
                        s_ps = ps.tile([128, twl], F32, tag="ps", name=f"s{w}_{h}")
                        for cc in range(nch):
                            st = cc == 0
                            sp = cc == nch - 1
                            pt, off = pts[cc]
                            psl = slice(off, off + twl)
                            nc.tensor.matmul(
                                y_ps[:], v_sb[:, cc * DH:(cc + 1) * DH],
                                pt[:, psl], start=st, stop=sp)
                            nc.tensor.matmul(
                                s_ps[:], ones_sb[:], pt[:, psl],
                                start=st, stop=sp)
                        rc = rcp.tile([128, twl], F32, tag="rc", name=f"rc{w}_{h}")
                        nc.vector.reciprocal(rc[:], s_ps[:])
                        nc.vector.tensor_mul(yt_sb[h][:, wsl], y_ps[:], rc[:])

                def emit_outproj_window(tw0, twl, split_out=False, rev=False):
                    jorder = range(twl // 128)
                    for j in (reversed(jorder) if rev else jorder):
                        t0 = tw0 + j * 128
                        po = [ps.tile([128, 512], F32, tag="ps",
                                      name=f"po{t0}_{e}") for e in range(4)]
                        for k in range(HPD):
                            for e in range(4):
                                nc.tensor.matmul(
                                    po[e][:],
                                    yt_sb[k][:, t0:t0 + 128],
                                    wp_sb[:, k * C + e * 512:k * C + (e + 1) * 512],
                                    start=(k == 0), stop=(k == HPD - 1))
                        ost = ostp.tile([128, C], F32, tag="ost", name=f"ost{t0}")
                        if split_out:
                            for e in range(4):
                                esl = slice(e * 512, (e + 1) * 512)
                                nc.any.tensor_copy(ost[:, esl], po[e][:])
                                nc.sync.dma_start(out[t0:t0 + 128, esl], ost[:, esl])
                        else:
                            for e in range(4):
                                nc.any.tensor_copy(ost[:, e * 512:(e + 1) * 512], po[e][:])
                            nc.sync.dma_start(out[t0:t0 + 128, :], ost[:])

                with (
                    tc.tile_pool(name="pt", bufs=10) as ptp,
                    tc.tile_pool(name="rc", bufs=2) as rcp,
                    tc.tile_pool(name="ost", bufs=3) as ostp,
                ):
                    SW = TQ // 2
                    for qq in range(2 * NQ):
                        emit_xt_loads(qq)
                    for i in range(2 * NQ):
                        if i >= 1:
                            w0 = (i - 1) * SW

                            def mid_a(w0=w0, i=i):
                                emit_v_transpose(i - 1)
                                emit_attn_head(w0, SW, 0)

                            def mid_b(w0=w0):
                                emit_attn_head(w0, SW, 1)
                            c0 = 2 if i <= 3 else 4
                            hooks = ((c0, mid_a), (12, mid_b))
                        else:
                            hooks = ()
                        emit_proj_subq(i, hooks=hooks)
                        if i >= 2:
                            emit_outproj_window((i - 2) * SW, SW)
                    emit_v_transpose(2 * NQ - 1)
                    emit_attn_head((2 * NQ - 1) * SW, SW, 0)
                    emit_outproj_window((2 * NQ - 2) * SW, SW, split_out=True)
                    emit_attn_head((2 * NQ - 1) * SW, SW, 1)
                    emit_outproj_window((2 * NQ - 1) * SW, SW, split_out=True, rev=True)

    nc.compile()
    return nc


_CACHE = {}


def _get_module():
    if "nc" not in _CACHE:
        nc = bacc.Bacc("TRN2", target_bir_lowering=False, debug=False)
        _CACHE["nc"] = _emit(nc)
    return _CACHE["nc"]


def _host_constants():
    if "consts" in _CACHE:
        return _CACHE["consts"]
    inv_freq = 1.0 / (ROPE_BASE ** (np.arange(0, DH, 2, dtype=np.float64) / DH))
    ang = np.outer(np.arange(T, dtype=np.float64), inv_freq)      # (T, 64)
    emb = np.concatenate([ang, ang], axis=-1)                     # (T, 128)
    cos = np.cos(emb).astype(np.float32)                          # (T, 128)
    sin = np.sin(emb).astype(np.float32)
    cosT = np.ascontiguousarray(cos.T)                            # (128, T)
    sinT = np.ascontiguousarray(sin.T)
    sign = np.where(np.arange(DH) < DH // 2, -1.0, 1.0).astype(np.float32)
    sinR = np.ascontiguousarray(sinT * sign[:, None])
    ones = np.ones((128, 128), dtype=np.float32)
    ident = np.eye(128, dtype=np.float32)
    _CACHE["consts"] = (cosT, sinR, ones, ident)
    return _CACHE["consts"]


def kernel(x, wq, wk, wv, wproj):
    x = np.asarray(x, dtype=np.float32)
    wq = np.asarray(wq, dtype=np.float32)
    wk = np.asarray(wk, dtype=np.float32)
    wv = np.asarray(wv, dtype=np.float32)
    wproj = np.asarray(wproj, dtype=np.float32)

    nc = _get_module()
    cosT, sinR, ones, ident = _host_constants()
    xT = np.ascontiguousarray(x[0].T)                             # (C, T)

    in_maps = []
    for d in range(N_CORES):
        h0 = HPD * d
        g = d // 2
        # wq columns for heads h0..h0+HPD-1 -> [128, NCT*HPD*DH] (c-tile major)
        wq_d = wq[:, h0 * DH:(h0 + HPD) * DH]                     # (C, HPD*DH)
        wq_l = np.ascontiguousarray(
            wq_d.reshape(NCT, 128, HPD * DH).transpose(1, 0, 2).reshape(128, -1))
        wk_d = wk[:, g * DH:(g + 1) * DH]
        wk_l = np.ascontiguousarray(
            wk_d.reshape(NCT, 128, DH).transpose(1, 0, 2).reshape(128, -1))
        wv_d = wv[:, g * DH:(g + 1) * DH]
        wv_l = np.ascontiguousarray(
            wv_d.reshape(NCT, 128, DH).transpose(1, 0, 2).reshape(128, -1))
        # wproj rows for our heads -> [128, HPD*C] (head-major free dim)
        wp_d = wproj[h0 * DH:(h0 + HPD) * DH, :]                  # (HPD*DH, C)
        wp_l = np.ascontiguousarray(
            wp_d.reshape(HPD, 128, C).transpose(1, 0, 2).reshape(128, -1))
        in_maps.append({
            "xT": xT, "wq": wq_l, "wk": wk_l, "wv": wv_l, "wp": wp_l,
            "cosT": cosT, "sinR": sinR, "ones": ones, "ident": ident,
        })

    res = run_bass_kernel_spmd(nc, in_maps, core_ids=list(range(N_CORES)))
    acc = res.results[0]["out"].astype(np.float32)
    for d in range(1, N_CORES):
        acc = acc + res.results[d]["out"].astype(np.float32)
    return acc.reshape(1, T, C)


# revision 39
# speedup vs baseline: 1.1488x; 1.0055x over previous
"""Trainium2 Bass kernel for CausalSelfAttentionModern (GQA + RoPE + causal SDPA).

Sharding: tensor-parallel over heads across 8 NeuronCores.
Device d owns q-heads {2d, 2d+1} and kv-head d//2.
Each device computes its heads' attention plus its slice of the output
projection (row-parallel); the host sums the 8 partial outputs.

All matmuls run as float32r (full-rate fp32 mode on the PE array).
"""

import numpy as np
import concourse.bacc as bacc
import concourse.tile as tile
import concourse.mybir as mybir
from concourse.bass_utils import run_bass_kernel_spmd

F32 = mybir.dt.float32
F32R = mybir.dt.float32r
EXP = mybir.ActivationFunctionType.Exp

# hardcoded problem shapes
T = 2048          # sequence length
C = 2048          # embedding dim
DH = 128          # head dim
NH = 16           # query heads
NKV = 4           # kv heads
N_CORES = 8
HPD = NH // N_CORES  # q-heads per device = 2
ROPE_BASE = 10000.0
SCALE = 1.0 / np.sqrt(DH)

NQ = 4            # t-quarters for projection phase
TQ = T // NQ      # 512
NW = 4            # attention tq windows
TW = T // NW      # 512
NCT = C // 128    # 16 contraction tiles
NTC = T // 128    # 16 token chunks


def _emit(nc):
    xT = nc.dram_tensor("xT", [C, T], F32R, kind="ExternalInput").ap()
    wq = nc.dram_tensor("wq", [128, NCT * HPD * DH], F32R, kind="ExternalInput").ap()
    wk = nc.dram_tensor("wk", [128, NCT * DH], F32R, kind="ExternalInput").ap()
    wv = nc.dram_tensor("wv", [128, NCT * DH], F32R, kind="ExternalInput").ap()
    wp = nc.dram_tensor("wp", [128, HPD * C], F32R, kind="ExternalInput").ap()
    cosT = nc.dram_tensor("cosT", [128, T], F32, kind="ExternalInput").ap()
    sinR = nc.dram_tensor("sinR", [128, T], F32, kind="ExternalInput").ap()
    ones = nc.dram_tensor("ones", [128, 128], F32R, kind="ExternalInput").ap()
    ident = nc.dram_tensor("ident", [128, 128], F32, kind="ExternalInput").ap()
    out = nc.dram_tensor("out", [T, C], F32, kind="ExternalOutput").ap()

    with tile.TileContext(nc) as tc:
        with (
            tc.tile_pool(name="cst", bufs=1) as cst,
            tc.tile_pool(name="ps", bufs=8, space="PSUM") as ps,
        ):
            # persistent SBUF tensors (DMAs emitted at first-use points below)
            cos_sb = cst.tile([128, T], F32, tag="cos")
            sin_sb = cst.tile([128, T], F32, tag="sin")
            ones_sb = cst.tile([128, 128], F32R, tag="ones")
            id_sb = cst.tile([128, 128], F32, tag="ident")
            wp_sb = cst.tile([128, HPD * C], F32R, tag="wp")

            qt_sb = [cst.tile([128, T], F32R, tag=f"qt{m}", name=f"qt{m}")
                     for m in range(HPD)]
            kt_sb = cst.tile([128, T], F32R, tag="kt")
            vtp_pool = None  # vt quarter tiles come from the rope pool
            v_sb = cst.tile([128, NTC * DH], F32R, tag="v")
            yt_sb = [cst.tile([128, T], F32R, tag=f"yt{m}", name=f"yt{m}")
                     for m in range(HPD)]

            # ---------------- projections + rope, per t-quarter ----------------
            with (
                tc.tile_pool(name="wqkv", bufs=1) as wqkv,
                tc.tile_pool(name="xts", bufs=5) as xts,
                tc.tile_pool(name="rope", bufs=1) as rope,
            ):
                wq_sb = wqkv.tile([128, NCT * HPD * DH], F32R, tag="wq")
                wk_sb = wqkv.tile([128, NCT * DH], F32R, tag="wk")
                wv_sb = wqkv.tile([128, NCT * DH], F32R, tag="wv")
                # weights on the ACT ring: c-tile groups so sems fire early
                for a, b in [(0, 1), (1, 2), (2, 4), (4, 8), (8, 16)]:
                    q1 = HPD * DH
                    nc.scalar.dma_start(wq_sb[:, a * q1:b * q1], wq[:, a * q1:b * q1])
                    nc.scalar.dma_start(wk_sb[:, a * DH:b * DH], wk[:, a * DH:b * DH])
                    nc.scalar.dma_start(wv_sb[:, a * DH:b * DH], wv[:, a * DH:b * DH])

                xt_tiles = {}
                vt_tiles = {}

                def emit_xt_loads(qq):
                    # per half (8 c-tiles x 256 tokens) strided load
                    SW = TQ // 2
                    tsl = slice(qq * SW, (qq + 1) * SW)
                    for half in range(2):
                        xt = xts.tile([128, 8 * SW], F32R, tag="xt",
                                      name=f"xtq{qq}_{half}")
                        c0 = half * 8
                        splits = [(0, 2), (2, 4), (4, 8)] if (qq == 0 and half == 0) else [(0, 8)]
                        for a, b in splits:
                            nc.sync.dma_start(
                                xt[:, a * SW:b * SW].rearrange("p (ct t) -> p ct t", t=SW),
                                xT[(c0 + a) * 128:(c0 + b) * 128, tsl].rearrange(
                                    "(ct p) t -> p ct t", p=128))
                        xt_tiles[(qq, half)] = xt

                def emit_proj_subq(i, hooks=()):
                    # one 256-token sub-quarter: psum tiles complete before rope
                    SW = TQ // 2
                    tsl = slice(i * SW, (i + 1) * SW)
                    pq = [ps.tile([128, SW], F32, tag="ps", name=f"pq{i}_{m}")
                          for m in range(HPD)]
                    pk = ps.tile([128, SW], F32, tag="ps", name=f"pk{i}")
                    pv = ps.tile([128, SW], F32, tag="ps", name=f"pv{i}")
                    hooks = dict(hooks)
                    for ct in range(NCT):
                        fn = hooks.pop(ct, None)
                        if fn is not None:
                            fn()
                        xt = xt_tiles[(i, ct // 8)]
                        xsl = slice((ct % 8) * SW, (ct % 8 + 1) * SW)
                        st = ct == 0
                        sp = ct == NCT - 1
                        for m in range(HPD):
                            nc.tensor.matmul(
                                pq[m][:],
                                wq_sb[:, (ct * HPD + m) * DH:(ct * HPD + m + 1) * DH],
                                xt[:, xsl], start=st, stop=sp)
                        nc.tensor.matmul(
                            pk[:], wk_sb[:, ct * DH:(ct + 1) * DH],
                            xt[:, xsl], start=st, stop=sp)
                        nc.tensor.matmul(
                            pv[:], wv_sb[:, ct * DH:(ct + 1) * DH],
                            xt[:, xsl], start=st, stop=sp)

                    if i == 0:
                        # constants needed from the rope/attention phases on
                        nc.scalar.dma_start(cos_sb[:], cosT[:])
                        nc.scalar.dma_start(sin_sb[:], sinR[:])
                        nc.scalar.dma_start(id_sb[:], ident[:])
                        nc.scalar.dma_start(ones_sb[:], ones[:])
                    if i == 1:
                        nc.scalar.dma_start(wp_sb[:], wp[:])

                    # rope: out = psum*cos + shift(psum)*sinR  (shift = rotate-half)
                    for psrc, dst in [(pq[0], qt_sb[0]), (pq[1], qt_sb[1]), (pk, kt_sb)]:
                        cr = rope.tile([128, SW], F32, tag="crope")
                        nc.vector.tensor_mul(cr[:], psrc[:], cos_sb[:, tsl])
                        ur = rope.tile([128, SW], F32, tag="urot")
                        nc.vector.tensor_mul(ur[0:64, :], psrc[64:128, :], sin_sb[0:64, tsl])
                        nc.vector.tensor_mul(ur[64:128, :], psrc[0:64, :], sin_sb[64:128, tsl])
                        nc.vector.tensor_add(dst[:, tsl], cr[:], ur[:])
                    # v: plain copy to SBUF (fp32, feeds PE transpose)
                    vt_q = rope.tile([128, SW], F32, tag="vtq", name=f"vtq{i}")
                    nc.scalar.copy(vt_q[:], pv[:])
                    vt_tiles[i] = vt_q

                def emit_v_transpose(i):
                    # transpose V^T -> V for sub-quarter i (2 token chunks)
                    SW = TQ // 2
                    pvt = ps.tile([128, SW], F32, tag="ps", name=f"pvt{i}")
                    for j in range(2):
                        nc.tensor.transpose(
                            pvt[:, j * 128:(j + 1) * 128],
                            vt_tiles[i][:, j * 128:(j + 1) * 128],
                            id_sb[:])
                    nc.vector.tensor_copy(v_sb[:, i * SW:(i + 1) * SW], pvt[:])

                def emit_attn_head(tw0, twl, h):
                    wsl = slice(tw0, tw0 + twl)
                    nch = (tw0 + twl) // 128
                    w = tw0 // 128  # first diagonal chunk index
                    if True:
                        # phase 1: scores -> exp -> causal-zero, decoupled from pV
                        # chunk PAIRS share one psum bank and one exp instruction
                        pts = []
                        for cp in range(0, nch, 2):
                            npair = min(2, nch - cp)
                            pw = npair * twl
                            sc_ps = ps.tile([128, pw], F32, tag="ps",
                                            name=f"sc{w}_{h}_{cp}")
                            for k2 in range(npair):
                                cc = cp + k2
                                nc.tensor.matmul(
                                    sc_ps[:, k2 * twl:(k2 + 1) * twl],
                                    kt_sb[:, cc * 128:(cc + 1) * 128],
                                    qt_sb[h][:, wsl], start=True, stop=True)
                            pt = ptp.tile([128, pw], F32R, tag="pt",
                                          name=f"pt{w}_{h}_{cp}")
                            nc.scalar.activation(pt[:], sc_ps[:], EXP,
                                                 scale=float(SCALE))
                            for k2 in range(npair):
                                cc = cp + k2
                                if cc >= w:
                                    # zero strictly-above-diagonal: keep tq >= tk
                                    nc.gpsimd.affine_select(
                                        out=pt[:, k2 * twl:(k2 + 1) * twl],
                                        in_=pt[:, k2 * twl:(k2 + 1) * twl],
                                        pattern=[[1, twl]],
                                        compare_op=mybir.AluOpType.is_ge, fill=0.0,
                                        base=tw0 - cc * 128, channel_multiplier=-1)
                                pts.append((pt, k2 * twl))
                        # phase 2: y^T += V^T-chunks @ probs, sums via ones-matmul
                        y_ps = ps.tile([128, twl], F32, tag="ps", name=f"y{w}_{h}")
                        s_ps = ps.tile([128, twl], F32, tag="ps", name=f"s{w}_{h}")
                        for cc in range(nch):
                            st = cc == 0
                            sp = cc == nch - 1
                            pt, off = pts[cc]
                            psl = slice(off, off + twl)
                            nc.tensor.matmul(
                                y_ps[:], v_sb[:, cc * DH:(cc + 1) * DH],
                                pt[:, psl], start=st, stop=sp)
                            nc.tensor.matmul(
                                s_ps[:], ones_sb[:], pt[:, psl],
                                start=st, stop=sp)
                        rc = rcp.tile([128, twl], F32, tag="rc", name=f"rc{w}_{h}")
                        nc.vector.reciprocal(rc[:], s_ps[:])
                        nc.vector.tensor_mul(yt_sb[h][:, wsl], y_ps[:], rc[:])

                def emit_outproj_window(tw0, twl, split_out=False, rev=False,
                                        per_e=False):
                    jorder = range(twl // 128)
                    for j in (reversed(jorder) if rev else jorder):
                        t0 = tw0 + j * 128
                        ost = ostp.tile([128, C], F32, tag="ost", name=f"ost{t0}")
                        if per_e:
                            # hold one psum bank at a time (weavable inside proj)
                            for e in range(4):
                                po = ps.tile([128, 512], F32, tag="ps",
                                             name=f"po{t0}_{e}")
                                for k in range(HPD):
                                    nc.tensor.matmul(
                                        po[:],
                                        yt_sb[k][:, t0:t0 + 128],
                                        wp_sb[:, k * C + e * 512:k * C + (e + 1) * 512],
                                        start=(k == 0), stop=(k == HPD - 1))
                                nc.any.tensor_copy(ost[:, e * 512:(e + 1) * 512], po[:])
                        else:
                            po = [ps.tile([128, 512], F32, tag="ps",
                                          name=f"po{t0}_{e}") for e in range(4)]
                            for k in range(HPD):
                                for e in range(4):
                                    nc.tensor.matmul(
                                        po[e][:],
                                        yt_sb[k][:, t0:t0 + 128],
                                        wp_sb[:, k * C + e * 512:k * C + (e + 1) * 512],
                                        start=(k == 0), stop=(k == HPD - 1))
                            for e in range(4):
                                nc.any.tensor_copy(ost[:, e * 512:(e + 1) * 512], po[e][:])
                        if split_out:
                            for e in range(4):
                                esl = slice(e * 512, (e + 1) * 512)
                                nc.sync.dma_start(out[t0:t0 + 128, esl], ost[:, esl])
                        else:
                            nc.sync.dma_start(out[t0:t0 + 128, :], ost[:])

                with (
                    tc.tile_pool(name="pt", bufs=10) as ptp,
                    tc.tile_pool(name="rc", bufs=2) as rcp,
                    tc.tile_pool(name="ost", bufs=3) as ostp,
                ):
                    SW = TQ // 2
                    for qq in range(2 * NQ):
                        emit_xt_loads(qq)
                    for i in range(2 * NQ):
                        if i >= 1:
                            w0 = (i - 1) * SW

                            def mid_a(w0=w0, i=i):
                                emit_v_transpose(i - 1)
                                emit_attn_head(w0, SW, 0)

                            def mid_b(w0=w0):
                                emit_attn_head(w0, SW, 1)
                            c0 = 2 if i <= 3 else 4
                            hooks = ((c0, mid_a), (12, mid_b))
                        else:
                            hooks = ()
                        emit_proj_subq(i, hooks=hooks)
                        if i >= 2:
                            emit_outproj_window((i - 2) * SW, SW, per_e=True)
                    emit_v_transpose(2 * NQ - 1)
                    emit_attn_head((2 * NQ - 1) * SW, SW, 0)
                    emit_outproj_window((2 * NQ - 2) * SW, SW, split_out=True, per_e=True)
                    emit_attn_head((2 * NQ - 1) * SW, SW, 1)
                    emit_outproj_window((2 * NQ - 1) * SW, SW, split_out=True, rev=True, per_e=True)

    nc.compile()
    return nc


_CACHE = {}


def _get_module():
    if "nc" not in _CACHE:
        nc = bacc.Bacc("TRN2", target_bir_lowering=False, debug=False)
        _CACHE["nc"] = _emit(nc)
    return _CACHE["nc"]


def _host_constants():
    if "consts" in _CACHE:
        return _CACHE["consts"]
    inv_freq = 1.0 / (ROPE_BASE ** (np.arange(0, DH, 2, dtype=np.float64) / DH))
    ang = np.outer(np.arange(T, dtype=np.float64), inv_freq)      # (T, 64)
    emb = np.concatenate([ang, ang], axis=-1)                     # (T, 128)
    cos = np.cos(emb).astype(np.float32)                          # (T, 128)
    sin = np.sin(emb).astype(np.float32)
    cosT = np.ascontiguousarray(cos.T)                            # (128, T)
    sinT = np.ascontiguousarray(sin.T)
    sign = np.where(np.arange(DH) < DH // 2, -1.0, 1.0).astype(np.float32)
    sinR = np.ascontiguousarray(sinT * sign[:, None])
    ones = np.ones((128, 128), dtype=np.float32)
    ident = np.eye(128, dtype=np.float32)
    _CACHE["consts"] = (cosT, sinR, ones, ident)
    return _CACHE["consts"]


def kernel(x, wq, wk, wv, wproj):
    x = np.asarray(x, dtype=np.float32)
    wq = np.asarray(wq, dtype=np.float32)
    wk = np.asarray(wk, dtype=np.float32)
    wv = np.asarray(wv, dtype=np.float32)
    wproj = np.asarray(wproj, dtype=np.float32)

    nc = _get_module()
    cosT, sinR, ones, ident = _host_constants()
    xT = np.ascontiguousarray(x[0].T)                             # (C, T)

    in_maps = []
    for d in range(N_CORES):
        h0 = HPD * d
        g = d // 2
        # wq columns for heads h0..h0+HPD-1 -> [128, NCT*HPD*DH] (c-tile major)
        wq_d = wq[:, h0 * DH:(h0 + HPD) * DH]                     # (C, HPD*DH)
        wq_l = np.ascontiguousarray(
            wq_d.reshape(NCT, 128, HPD * DH).transpose(1, 0, 2).reshape(128, -1))
        wk_d = wk[:, g * DH:(g + 1) * DH]
        wk_l = np.ascontiguousarray(
            wk_d.reshape(NCT, 128, DH).transpose(1, 0, 2).reshape(128, -1))
        wv_d = wv[:, g * DH:(g + 1) * DH]
        wv_l = np.ascontiguousarray(
            wv_d.reshape(NCT, 128, DH).transpose(1, 0, 2).reshape(128, -1))
        # wproj rows for our heads -> [128, HPD*C] (head-major free dim)
        wp_d = wproj[h0 * DH:(h0 + HPD) * DH, :]                  # (HPD*DH, C)
        wp_l = np.ascontiguousarray(
            wp_d.reshape(HPD, 128, C).transpose(1, 0, 2).reshape(128, -1))
        in_maps.append({
            "xT": xT, "wq": wq_l, "wk": wk_l, "wv": wv_l, "wp": wp_l,
            "cosT": cosT, "sinR": sinR, "ones": ones, "ident": ident,
        })

    res = run_bass_kernel_spmd(nc, in_maps, core_ids=list(range(N_CORES)))
    acc = res.results[0]["out"].astype(np.float32)
    for d in range(1, N_CORES):
        acc = acc + res.results[d]["out"].astype(np.float32)
    return acc.reshape(1, T, C)


# revision 41
# speedup vs baseline: 1.1552x; 1.0056x over previous
"""Trainium2 Bass kernel for CausalSelfAttentionModern (GQA + RoPE + causal SDPA).

Sharding: tensor-parallel over heads across 8 NeuronCores.
Device d owns q-heads {2d, 2d+1} and kv-head d//2.
Each device computes its heads' attention plus its slice of the output
projection (row-parallel); the host sums the 8 partial outputs.

All matmuls run as float32r (full-rate fp32 mode on the PE array).
"""

import numpy as np
import concourse.bacc as bacc
import concourse.tile as tile
import concourse.mybir as mybir
from concourse.bass_utils import run_bass_kernel_spmd

F32 = mybir.dt.float32
F32R = mybir.dt.float32r
EXP = mybir.ActivationFunctionType.Exp

# hardcoded problem shapes
T = 2048          # sequence length
C = 2048          # embedding dim
DH = 128          # head dim
NH = 16           # query heads
NKV = 4           # kv heads
N_CORES = 8
HPD = NH // N_CORES  # q-heads per device = 2
ROPE_BASE = 10000.0
SCALE = 1.0 / np.sqrt(DH)

NQ = 4            # t-quarters for projection phase
TQ = T // NQ      # 512
NW = 4            # attention tq windows
TW = T // NW      # 512
NCT = C // 128    # 16 contraction tiles
NTC = T // 128    # 16 token chunks


def _emit(nc):
    xT = nc.dram_tensor("xT", [C, T], F32R, kind="ExternalInput").ap()
    wq = nc.dram_tensor("wq", [128, NCT * HPD * DH], F32R, kind="ExternalInput").ap()
    wk = nc.dram_tensor("wk", [128, NCT * DH], F32R, kind="ExternalInput").ap()
    wv = nc.dram_tensor("wv", [128, NCT * DH], F32R, kind="ExternalInput").ap()
    wp = nc.dram_tensor("wp", [128, HPD * C], F32R, kind="ExternalInput").ap()
    cosT = nc.dram_tensor("cosT", [128, T], F32, kind="ExternalInput").ap()
    sinR = nc.dram_tensor("sinR", [128, T], F32, kind="ExternalInput").ap()
    ones = nc.dram_tensor("ones", [128, 128], F32R, kind="ExternalInput").ap()
    ident = nc.dram_tensor("ident", [128, 128], F32, kind="ExternalInput").ap()
    out = nc.dram_tensor("out", [T, C], F32, kind="ExternalOutput").ap()

    with tile.TileContext(nc) as tc:
        with (
            tc.tile_pool(name="cst", bufs=1) as cst,
            tc.tile_pool(name="ps", bufs=8, space="PSUM") as ps,
        ):
            # persistent SBUF tensors (DMAs emitted at first-use points below)
            cos_sb = cst.tile([128, T], F32, tag="cos")
            sin_sb = cst.tile([128, T], F32, tag="sin")
            ones_sb = cst.tile([128, 128], F32R, tag="ones")
            id_sb = cst.tile([128, 128], F32, tag="ident")
            wp_sb = cst.tile([128, HPD * C], F32R, tag="wp")

            qt_sb = [cst.tile([128, T], F32R, tag=f"qt{m}", name=f"qt{m}")
                     for m in range(HPD)]
            kt_sb = cst.tile([128, T], F32R, tag="kt")
            vtp_pool = None  # vt quarter tiles come from the rope pool
            v_sb = cst.tile([128, NTC * DH], F32R, tag="v")
            yt_sb = [cst.tile([128, T], F32R, tag=f"yt{m}", name=f"yt{m}")
                     for m in range(HPD)]

            # ---------------- projections + rope, per t-quarter ----------------
            with (
                tc.tile_pool(name="wqkv", bufs=1) as wqkv,
                tc.tile_pool(name="xts", bufs=5) as xts,
                tc.tile_pool(name="rope", bufs=1) as rope,
            ):
                wq_sb = wqkv.tile([128, NCT * HPD * DH], F32R, tag="wq")
                wk_sb = wqkv.tile([128, NCT * DH], F32R, tag="wk")
                wv_sb = wqkv.tile([128, NCT * DH], F32R, tag="wv")
                # weights on the ACT ring: c-tile groups so sems fire early
                for a, b in [(0, 1), (1, 2), (2, 4), (4, 8), (8, 16)]:
                    q1 = HPD * DH
                    nc.scalar.dma_start(wq_sb[:, a * q1:b * q1], wq[:, a * q1:b * q1])
                    nc.scalar.dma_start(wk_sb[:, a * DH:b * DH], wk[:, a * DH:b * DH])
                    nc.scalar.dma_start(wv_sb[:, a * DH:b * DH], wv[:, a * DH:b * DH])

                xt_tiles = {}
                vt_tiles = {}

                def emit_xt_loads(qq):
                    # per half (8 c-tiles x 256 tokens) strided load
                    SW = TQ // 2
                    tsl = slice(qq * SW, (qq + 1) * SW)
                    for half in range(2):
                        xt = xts.tile([128, 8 * SW], F32R, tag="xt",
                                      name=f"xtq{qq}_{half}")
                        c0 = half * 8
                        splits = [(0, 2), (2, 4), (4, 8)] if (qq == 0 and half == 0) else [(0, 8)]
                        for a, b in splits:
                            nc.sync.dma_start(
                                xt[:, a * SW:b * SW].rearrange("p (ct t) -> p ct t", t=SW),
                                xT[(c0 + a) * 128:(c0 + b) * 128, tsl].rearrange(
                                    "(ct p) t -> p ct t", p=128))
                        xt_tiles[(qq, half)] = xt

                def emit_proj_subq(i, hooks=()):
                    # one 256-token sub-quarter: psum tiles complete before rope
                    SW = TQ // 2
                    tsl = slice(i * SW, (i + 1) * SW)
                    pq = [ps.tile([128, SW], F32, tag="ps", name=f"pq{i}_{m}")
                          for m in range(HPD)]
                    pk = ps.tile([128, SW], F32, tag="ps", name=f"pk{i}")
                    pv = ps.tile([128, SW], F32, tag="ps", name=f"pv{i}")
                    hooks = dict(hooks)
                    for ct in range(NCT):
                        fn = hooks.pop(ct, None)
                        if fn is not None:
                            fn()
                        xt = xt_tiles[(i, ct // 8)]
                        xsl = slice((ct % 8) * SW, (ct % 8 + 1) * SW)
                        st = ct == 0
                        sp = ct == NCT - 1
                        for m in range(HPD):
                            nc.tensor.matmul(
                                pq[m][:],
                                wq_sb[:, (ct * HPD + m) * DH:(ct * HPD + m + 1) * DH],
                                xt[:, xsl], start=st, stop=sp)
                        nc.tensor.matmul(
                            pk[:], wk_sb[:, ct * DH:(ct + 1) * DH],
                            xt[:, xsl], start=st, stop=sp)
                        nc.tensor.matmul(
                            pv[:], wv_sb[:, ct * DH:(ct + 1) * DH],
                            xt[:, xsl], start=st, stop=sp)

                    if i == 0:
                        # constants needed from the rope/attention phases on
                        # (only 64 rows of each rope table come from HBM; the
                        # other half is derived: cos repeats, sin negates)
                        nc.scalar.dma_start(cos_sb[0:64, :], cosT[0:64, :])
                        nc.scalar.dma_start(sin_sb[0:64, :], sinR[0:64, :])
                        nc.scalar.copy(cos_sb[64:128, :], cos_sb[0:64, :])
                        nc.scalar.mul(sin_sb[64:128, :], sin_sb[0:64, :], -1.0)
                        nc.scalar.dma_start(id_sb[:], ident[:])
                        nc.scalar.dma_start(ones_sb[:], ones[:])
                    if i == 1:
                        nc.scalar.dma_start(wp_sb[:], wp[:])

                    # rope: out = psum*cos + shift(psum)*sinR  (shift = rotate-half)
                    for psrc, dst in [(pq[0], qt_sb[0]), (pq[1], qt_sb[1]), (pk, kt_sb)]:
                        cr = rope.tile([128, SW], F32, tag="crope")
                        nc.vector.tensor_mul(cr[:], psrc[:], cos_sb[:, tsl])
                        ur = rope.tile([128, SW], F32, tag="urot")
                        nc.vector.tensor_mul(ur[0:64, :], psrc[64:128, :], sin_sb[0:64, tsl])
                        nc.vector.tensor_mul(ur[64:128, :], psrc[0:64, :], sin_sb[64:128, tsl])
                        nc.vector.tensor_add(dst[:, tsl], cr[:], ur[:])
                    # v: plain copy to SBUF (fp32, feeds PE transpose)
                    vt_q = rope.tile([128, SW], F32, tag="vtq", name=f"vtq{i}")
                    nc.scalar.copy(vt_q[:], pv[:])
                    vt_tiles[i] = vt_q

                def emit_v_transpose(i):
                    # transpose V^T -> V for sub-quarter i (2 token chunks)
                    SW = TQ // 2
                    pvt = ps.tile([128, SW], F32, tag="ps", name=f"pvt{i}")
                    for j in range(2):
                        nc.tensor.transpose(
                            pvt[:, j * 128:(j + 1) * 128],
                            vt_tiles[i][:, j * 128:(j + 1) * 128],
                            id_sb[:])
                    nc.vector.tensor_copy(v_sb[:, i * SW:(i + 1) * SW], pvt[:])

                def emit_attn_head(tw0, twl, h):
                    wsl = slice(tw0, tw0 + twl)
                    nch = (tw0 + twl) // 128
                    w = tw0 // 128  # first diagonal chunk index
                    if True:
                        # phase 1: scores -> exp -> causal-zero, decoupled from pV
                        # chunk PAIRS share one psum bank and one exp instruction
                        pts = []
                        for cp in range(0, nch, 2):
                            npair = min(2, nch - cp)
                            pw = npair * twl
                            sc_ps = ps.tile([128, pw], F32, tag="ps",
                                            name=f"sc{w}_{h}_{cp}")
                            for k2 in range(npair):
                                cc = cp + k2
                                nc.tensor.matmul(
                                    sc_ps[:, k2 * twl:(k2 + 1) * twl],
                                    kt_sb[:, cc * 128:(cc + 1) * 128],
                                    qt_sb[h][:, wsl], start=True, stop=True)
                            pt = ptp.tile([128, pw], F32R, tag="pt",
                                          name=f"pt{w}_{h}_{cp}")
                            nc.scalar.activation(pt[:], sc_ps[:], EXP,
                                                 scale=float(SCALE))
                            for k2 in range(npair):
                                cc = cp + k2
                                if cc >= w:
                                    # zero strictly-above-diagonal: keep tq >= tk
                                    nc.gpsimd.affine_select(
                                        out=pt[:, k2 * twl:(k2 + 1) * twl],
                                        in_=pt[:, k2 * twl:(k2 + 1) * twl],
                                        pattern=[[1, twl]],
                                        compare_op=mybir.AluOpType.is_ge, fill=0.0,
                                        base=tw0 - cc * 128, channel_multiplier=-1)
                                pts.append((pt, k2 * twl))
                        # phase 2: y^T += V^T-chunks @ probs, sums via ones-matmul
                        y_ps = ps.tile([128, twl], F32, tag="ps", name=f"y{w}_{h}")
                        s_ps = ps.tile([128, twl], F32, tag="ps", name=f"s{w}_{h}")
                        for cc in range(nch):
                            st = cc == 0
                            sp = cc == nch - 1
                            pt, off = pts[cc]
                            psl = slice(off, off + twl)
                            nc.tensor.matmul(
                                y_ps[:], v_sb[:, cc * DH:(cc + 1) * DH],
                                pt[:, psl], start=st, stop=sp)
                            nc.tensor.matmul(
                                s_ps[:], ones_sb[:], pt[:, psl],
                                start=st, stop=sp)
                        rc = rcp.tile([128, twl], F32, tag="rc", name=f"rc{w}_{h}")
                        nc.vector.reciprocal(rc[:], s_ps[:])
                        nc.vector.tensor_mul(yt_sb[h][:, wsl], y_ps[:], rc[:])

                def emit_outproj_window(tw0, twl, split_out=False, rev=False,
                                        per_e=False):
                    jorder = range(twl // 128)
                    for j in (reversed(jorder) if rev else jorder):
                        t0 = tw0 + j * 128
                        ost = ostp.tile([128, C], F32, tag="ost", name=f"ost{t0}")
                        if per_e:
                            # hold one psum bank at a time (weavable inside proj)
                            for e in range(4):
                                po = ps.tile([128, 512], F32, tag="ps",
                                             name=f"po{t0}_{e}")
                                for k in range(HPD):
                                    nc.tensor.matmul(
                                        po[:],
                                        yt_sb[k][:, t0:t0 + 128],
                                        wp_sb[:, k * C + e * 512:k * C + (e + 1) * 512],
                                        start=(k == 0), stop=(k == HPD - 1))
                                nc.any.tensor_copy(ost[:, e * 512:(e + 1) * 512], po[:])
                        else:
                            po = [ps.tile([128, 512], F32, tag="ps",
                                          name=f"po{t0}_{e}") for e in range(4)]
                            for k in range(HPD):
                                for e in range(4):
                                    nc.tensor.matmul(
                                        po[e][:],
                                        yt_sb[k][:, t0:t0 + 128],
                                        wp_sb[:, k * C + e * 512:k * C + (e + 1) * 512],
                                        start=(k == 0), stop=(k == HPD - 1))
                            for e in range(4):
                                nc.any.tensor_copy(ost[:, e * 512:(e + 1) * 512], po[e][:])
                        if split_out:
                            for e in range(4):
                                esl = slice(e * 512, (e + 1) * 512)
                                nc.sync.dma_start(out[t0:t0 + 128, esl], ost[:, esl])
                        else:
                            nc.sync.dma_start(out[t0:t0 + 128, :], ost[:])

                with (
                    tc.tile_pool(name="pt", bufs=10) as ptp,
                    tc.tile_pool(name="rc", bufs=2) as rcp,
                    tc.tile_pool(name="ost", bufs=3) as ostp,
                ):
                    SW = TQ // 2
                    for qq in range(2 * NQ):
                        emit_xt_loads(qq)
                    for i in range(2 * NQ):
                        if i >= 1:
                            w0 = (i - 1) * SW

                            def mid_a(w0=w0, i=i):
                                emit_v_transpose(i - 1)
                                emit_attn_head(w0, SW, 0)

                            def mid_b(w0=w0):
                                emit_attn_head(w0, SW, 1)
                            c0 = 2 if i <= 3 else 4
                            hooks = ((c0, mid_a), (12, mid_b))
                        else:
                            hooks = ()
                        emit_proj_subq(i, hooks=hooks)
                        if i >= 2:
                            emit_outproj_window((i - 2) * SW, SW, per_e=True)
                    emit_v_transpose(2 * NQ - 1)
                    emit_attn_head((2 * NQ - 1) * SW, SW, 0)
                    emit_outproj_window((2 * NQ - 2) * SW, SW, split_out=True, per_e=True)
                    emit_attn_head((2 * NQ - 1) * SW, SW, 1)
                    emit_outproj_window((2 * NQ - 1) * SW, SW, split_out=True, rev=True, per_e=True)

    nc.compile()
    return nc


_CACHE = {}


def _get_module():
    if "nc" not in _CACHE:
        nc = bacc.Bacc("TRN2", target_bir_lowering=False, debug=False)
        _CACHE["nc"] = _emit(nc)
    return _CACHE["nc"]


def _host_constants():
    if "consts" in _CACHE:
        return _CACHE["consts"]
    inv_freq = 1.0 / (ROPE_BASE ** (np.arange(0, DH, 2, dtype=np.float64) / DH))
    ang = np.outer(np.arange(T, dtype=np.float64), inv_freq)      # (T, 64)
    emb = np.concatenate([ang, ang], axis=-1)                     # (T, 128)
    cos = np.cos(emb).astype(np.float32)                          # (T, 128)
    sin = np.sin(emb).astype(np.float32)
    cosT = np.ascontiguousarray(cos.T)                            # (128, T)
    sinT = np.ascontiguousarray(sin.T)
    sign = np.where(np.arange(DH) < DH // 2, -1.0, 1.0).astype(np.float32)
    sinR = np.ascontiguousarray(sinT * sign[:, None])
    ones = np.ones((128, 128), dtype=np.float32)
    ident = np.eye(128, dtype=np.float32)
    _CACHE["consts"] = (cosT, sinR, ones, ident)
    return _CACHE["consts"]


def kernel(x, wq, wk, wv, wproj):
    x = np.asarray(x, dtype=np.float32)
    wq = np.asarray(wq, dtype=np.float32)
    wk = np.asarray(wk, dtype=np.float32)
    wv = np.asarray(wv, dtype=np.float32)
    wproj = np.asarray(wproj, dtype=np.float32)

    nc = _get_module()
    cosT, sinR, ones, ident = _host_constants()
    xT = np.ascontiguousarray(x[0].T)                             # (C, T)

    in_maps = []
    for d in range(N_CORES):
        h0 = HPD * d
        g = d // 2
        # wq columns for heads h0..h0+HPD-1 -> [128, NCT*HPD*DH] (c-tile major)
        wq_d = wq[:, h0 * DH:(h0 + HPD) * DH]                     # (C, HPD*DH)
        wq_l = np.ascontiguousarray(
            wq_d.reshape(NCT, 128, HPD * DH).transpose(1, 0, 2).reshape(128, -1))
        wk_d = wk[:, g * DH:(g + 1) * DH]
        wk_l = np.ascontiguousarray(
            wk_d.reshape(NCT, 128, DH).transpose(1, 0, 2).reshape(128, -1))
        wv_d = wv[:, g * DH:(g + 1) * DH]
        wv_l = np.ascontiguousarray(
            wv_d.reshape(NCT, 128, DH).transpose(1, 0, 2).reshape(128, -1))
        # wproj rows for our heads -> [128, HPD*C] (head-major free dim)
        wp_d = wproj[h0 * DH:(h0 + HPD) * DH, :]                  # (HPD*DH, C)
        wp_l = np.ascontiguousarray(
            wp_d.reshape(HPD, 128, C).transpose(1, 0, 2).reshape(128, -1))
        in_maps.append({
            "xT": xT, "wq": wq_l, "wk": wk_l, "wv": wv_l, "wp": wp_l,
            "cosT": cosT, "sinR": sinR, "ones": ones, "ident": ident,
        })

    res = run_bass_kernel_spmd(nc, in_maps, core_ids=list(range(N_CORES)))
    acc = res.results[0]["out"].astype(np.float32)
    for d in range(1, N_CORES):
        acc = acc + res.results[d]["out"].astype(np.float32)
    return acc.reshape(1, T, C)
